# revision 21
# baseline (speedup 1.0000x reference)
"""BiMamba encoder layer on 8 Trainium2 NeuronCores (Bass/Tile, SPMD).

Sharding: core c = (batch b, direction dir, d_inner-half dh), c = b*4 + dir*2 + dh.
All 8 cores run an IDENTICAL program; per-core behavior comes only from input
data (host pre-sliced/pre-flipped weights and activations) and collective
replica groups.

v2: all DRAM bounce buffers are FEATURE-MAJOR so every DMA moves contiguous
2KB rows (v1 used token-major transposing DMAs: 2.6M 4-byte packets, 96% DMA
time). The b-direction time-flip is done on-chip with reversed-AP DVE copies;
B/C state broadcasts use gpsimd partition_broadcast instead of DRAM broadcast
DMAs; the final [D_MODEL, TAIL] -> [TAIL, D_MODEL] transpose happens on host.

Pipeline per core (512 of 1024 d_inner channels, full L for one (b, dir)):
  A) in_proj (PE, bf16) -> causal conv (STT taps) -> silu; silu(z) saved;
     x_proj partial (PE) -> per-chunk pair AllReduce (overlapped)
  B) dt (PE) -> softplus (ACT) -> dA = exp(A*delta) (ACT) -> dBu (merged DVE
     mult w/ stride-0 broadcast) -> 16 hardware scans (DVE, in-place) ->
     y = C.h (DVE mul + tree) -> gate -> out_proj partial (PE) -> xm
     feature-major [2, 512, 1024] (l'-half major)
  C) pair ReduceScatter (l' halves) -> cross AllGather
  D) tail: x_f branch (r1/LN1/FFN1/LN2/FFN2) reads own rs_out (overlaps AG);
     x_b branch (LN3/LN4/sum) reads ag_out[1] reversed on-chip. LN stats are
     broadcast via ones-matmul on PE (no gpsimd -> no queue conflict with AG).

Host: out[b, th half] = f-direction cores' [D_MODEL, TAIL] output, transposed.
"""
import sys
import os

sys.path.insert(0, '/opt/trn_rl_repo')

import numpy as np
import ml_dtypes

import concourse.bass as bass
import concourse.mybir as mybir
import concourse.tile as tile
from concourse import bacc
from concourse.bass_utils import run_bass_kernel_spmd
from concourse.bass import ds, ts

f32 = mybir.dt.float32
bf16 = mybir.dt.bfloat16
Alu = mybir.AluOpType
AFT = mybir.ActivationFunctionType

P = 128
D_MODEL = 512
D_INNER = 1024
DH = 512            # d_inner channels per core
NB = DH // P        # 4 channel blocks per core
NSTATE = 16
DT_RANK = 32
DBC = DT_RANK + 2 * NSTATE   # 64
D_CONV = 4
D_FF = 1024
NCORES = 8
LN_EPS = 1e-5

PAIRS = [[0, 1], [2, 3], [4, 5], [6, 7]]
CROSS = [[0, 3], [1, 2], [4, 7], [5, 6]]


def build_program(L, T, dbg=False):
    """Emit the SPMD program for sequence length L, phase chunk T."""
    NCH = L // T
    TAIL = L // 2
    MB = D_MODEL // P   # 4 blocks of d_model
    FB = D_FF // P      # 8 blocks of d_ff
    NT = NSTATE * T

    nc = bacc.Bacc('TRN2', target_bir_lowering=False, debug=False,
                   num_devices=NCORES)

    def din(name, shape, dt=f32):
        return nc.dram_tensor(name, shape, dt, kind='ExternalInput')

    xT16 = din('xT16', [D_MODEL, L], bf16)
    in_wT = din('in_wT', [D_MODEL, 2 * DH], bf16)       # K x (xi|z)
    conv_w = din('conv_w', [P, NB * D_CONV])
    conv_b = din('conv_b', [P, NB])
    xproj_wT = din('xproj_wT', [DH, DBC], bf16)
    dt_wT = din('dt_wT', [DT_RANK, DH], bf16)
    dt_b = din('dt_b', [P, NB])
    A_sc = din('A_sc', [P, NB * NSTATE])
    D_in = din('D_in', [P, NB])
    out_wT = din('out_wT', [DH, D_MODEL], bf16)
    f1w1 = din('f1w1', [D_MODEL, D_FF], bf16)
    f1b1 = din('f1b1', [P, D_FF // P])
    f1w2 = din('f1w2', [D_FF, D_MODEL], bf16)
    f1b2 = din('f1b2', [P, D_MODEL // P])
    f2w1 = din('f2w1', [D_MODEL, D_FF], bf16)
    f2b1 = din('f2b1', [P, D_FF // P])
    f2w2 = din('f2w2', [D_FF, D_MODEL], bf16)
    f2b2 = din('f2b2', [P, D_MODEL // P])
    lnp = din('lnp', [P, 8 * (D_MODEL // P)])           # ln1..4 w,b
    tail_x = din('tail_x', [D_MODEL, TAIL])
    out_t = nc.dram_tensor('out', [D_MODEL, TAIL], f32, kind='ExternalOutput')
    if dbg:
        dbg_xic = nc.dram_tensor('dbg_xic', [DH, L], f32, kind='ExternalOutput')
        dbg_sz = nc.dram_tensor('dbg_sz', [DH, L], f32, kind='ExternalOutput')
        dbg_dbc = nc.dram_tensor('dbg_dbc', [DBC, L], f32, kind='ExternalOutput')
        dbg_delta = nc.dram_tensor('dbg_delta', [P, T], f32, kind='ExternalOutput')
        dbg_h = nc.dram_tensor('dbg_h', [P, NSTATE * T], f32, kind='ExternalOutput')
        dbg_yg = nc.dram_tensor('dbg_yg', [P, T], f32, kind='ExternalOutput')
        dbg_xm = nc.dram_tensor('dbg_xm', [D_MODEL, L], f32, kind='ExternalOutput')
        dbg_rs = nc.dram_tensor('dbg_rs', [D_MODEL, TAIL], f32, kind='ExternalOutput')
        dbg_ag = nc.dram_tensor('dbg_ag', [2 * D_MODEL, TAIL], f32,
                                kind='ExternalOutput')
        dbg_tails = {_n: nc.dram_tensor('dbg_' + _n, [D_MODEL, TAIL], f32,
                                        kind='ExternalOutput')
                     for _n in ('r1', 't1', 'ff1', 't2', 'r3', 't3', 'ff2', 't4')}

    with tile.TileContext(nc) as tc:
        dram_cm = tc.tile_pool(name='dram', bufs=1, space='DRAM')
        dram = dram_cm.__enter__()
        dbc_bo = dram.tile([NCH, DBC, T], f32)
        dbc_ar = dram.tile([NCH, DBC, T], f32)
        dbcBC = dram.tile([NCH, 2 * NSTATE, T], bf16)
        xm_bo = dram.tile([2, D_MODEL, TAIL], f32)
        rs_out = dram.tile([D_MODEL, TAIL], f32)
        ag_out = dram.tile([2, D_MODEL, TAIL], f32)

        with tc.tile_pool(name='pers', bufs=1) as pers:
            # persistent SBUF (live through phases A-B; small ones to D)
            xpw_sb = [pers.tile([P, DBC], bf16, name=f'xpw{k}')
                      for k in range(NB)]
            for k in range(NB):
                nc.sync.dma_start(xpw_sb[k][:], xproj_wT[ts(k, P), :])
            dtw_sb = pers.tile([DT_RANK, DH], bf16)
            nc.sync.dma_start(dtw_sb[:], dt_wT[:])
            outw_sb = [pers.tile([P, D_MODEL], bf16, name=f'outw{k}') for k in range(NB)]
            for k in range(NB):
                nc.sync.dma_start(outw_sb[k][:], out_wT[ts(k, P), :])
            cw_sb = pers.tile([P, NB * D_CONV], f32)
            nc.sync.dma_start(cw_sb[:], conv_w[:])
            cb_sb = pers.tile([P, NB], f32)
            nc.sync.dma_start(cb_sb[:], conv_b[:])
            dtb_sb = pers.tile([P, NB], f32)
            nc.sync.dma_start(dtb_sb[:], dt_b[:])
            asc_sb = pers.tile([P, NB * NSTATE], f32)
            nc.sync.dma_start(asc_sb[:], A_sc[:])
            d_sb = pers.tile([P, NB], f32)
            nc.sync.dma_start(d_sb[:], D_in[:])

            xi_c = [pers.tile([P, L], bf16, name=f'xic{k}') for k in range(NB)]
            sz = [pers.tile([P, L], bf16, name=f'sz{k}') for k in range(NB)]
            halo = [pers.tile([P, D_CONV - 1], bf16, name=f'halo{k}') for k in range(NB)]
            for k in range(NB):
                nc.vector.memset(halo[k][:], 0.0)
            dbc16 = pers.tile([DBC, L], bf16)

            # ---------------- Phase A ----------------
            with tc.tile_pool(name='pax', bufs=1) as pax, \
                 tc.tile_pool(name='pa', bufs=3) as pa, \
                 tc.tile_pool(name='pa_ps', bufs=4, space='PSUM') as pa_ps:
                x_sb = [pax.tile([P, L], bf16, name=f'x_sb{k}') for k in range(MB)]
                for k in range(MB):
                    nc.sync.dma_start(x_sb[k][:], xT16[ts(k, P), :])
                inw_sb = [pax.tile([P, 2 * DH], bf16, name=f'inw{k}')
                          for k in range(MB)]
                for k in range(MB):
                    nc.sync.dma_start(inw_sb[k][:], in_wT[ts(k, P), :])
                for ci in range(NCH):
                    tsl = ds(ci * T, T)
                    for mb in range(2 * NB):   # 4 xi blocks then 4 z blocks
                        ps_t = pa_ps.tile([P, T], f32, name='inproj')
                        for kb in range(MB):
                            nc.tensor.matmul(
                                ps_t[:], inw_sb[kb][:, ts(mb, P)],
                                x_sb[kb][:, tsl],
                                start=(kb == 0), stop=(kb == MB - 1))
                        if mb < NB:
                            db = mb
                            xr = pa.tile([P, D_CONV - 1 + T], bf16, name='xr',
                                         tag='xr')
                            nc.vector.tensor_copy(xr[:, 0:D_CONV - 1], halo[db][:])
                            nc.scalar.activation(xr[:, D_CONV - 1:], ps_t[:],
                                                 AFT.Copy)
                            nc.vector.tensor_copy(halo[db][:], xr[:, T:])
                            # causal conv: tap 0 with bias, then taps 1..3
                            xc = pa.tile([P, T], bf16, name='xc', tag='xc')
                            nc.vector.tensor_scalar(
                                xc[:], xr[:, 0:T],
                                cw_sb[:, db * D_CONV:db * D_CONV + 1],
                                cb_sb[:, db:db + 1],
                                op0=Alu.mult, op1=Alu.add)
                            for k in range(1, D_CONV):
                                nc.vector.scalar_tensor_tensor(
                                    xc[:], xr[:, k:k + T],
                                    cw_sb[:, db * D_CONV + k:db * D_CONV + k + 1],
                                    xc[:], op0=Alu.mult, op1=Alu.add)
                            nc.scalar.activation(xi_c[db][:, tsl], xc[:], AFT.Silu)
                        else:
                            db = mb - NB
                            nc.scalar.activation(sz[db][:, tsl], ps_t[:], AFT.Silu)
                    # x_proj partial for this chunk -> per-chunk AllReduce
                    ps_x = pa_ps.tile([P, T], f32, name='xproj')
                    for kb in range(NB):
                        nc.tensor.matmul(
                            ps_x[0:DBC, :], xpw_sb[kb][:],
                            xi_c[kb][:, tsl],
                            start=(kb == 0), stop=(kb == NB - 1))
                    dbc_p = pa.tile([DBC, T], f32, name='dbcp', tag='dbcp')
                    nc.scalar.activation(dbc_p[:], ps_x[0:DBC, :], AFT.Copy)
                    nc.sync.dma_start(dbc_bo[ci], dbc_p[:])
                    nc.gpsimd.collective_compute(
                        'AllReduce', Alu.add, replica_groups=PAIRS,
                        ins=[dbc_bo[ci].opt()], outs=[dbc_ar[ci].opt()])



            if dbg:
                with tc.tile_pool(name='dbgp', bufs=1) as dbgp:
                    for k in range(NB):
                        t_ = dbgp.tile([P, L], f32, name='dbgt', tag='dbgt')
                        nc.vector.tensor_copy(t_[:], xi_c[k][:])
                        nc.sync.dma_start(dbg_xic[ts(k, P), :], t_[:])
                        t2_ = dbgp.tile([P, L], f32, name='dbgt2', tag='dbgt')
                        nc.vector.tensor_copy(t2_[:], sz[k][:])
                        nc.sync.dma_start(dbg_sz[ts(k, P), :], t2_[:])
                    t3_ = dbgp.tile([DBC, L], f32, name='dbgt3', tag='dbgt3')
                    nc.vector.tensor_copy(t3_[:], dbc16[:])
                    nc.sync.dma_start(dbg_dbc[:], t3_[:])

            # ---------------- Phase B ----------------
            state = [pers.tile([P, NSTATE], f32, name=f'st{k}') for k in range(NB)]
            with tc.tile_pool(name='pb', bufs=2) as pb, \
                 tc.tile_pool(name='pb0', bufs=2) as pb0, \
                 tc.tile_pool(name='pbh', bufs=2) as pbh, \
                 tc.tile_pool(name='pbr', bufs=2) as pbr, \
                 tc.tile_pool(name='pb_ps', bufs=2, space='PSUM') as pb_ps, \
                 tc.tile_pool(name='po_ps', bufs=1, space='PSUM') as po_ps:
                def consume(h, ci, db, tsl, ps_o):
                    half = NSTATE
                    while half > 1:
                        half //= 2
                        nc.vector.tensor_tensor(
                            h[:, 0:half * T], h[:, 0:half * T],
                            h[:, half * T:2 * half * T], op=Alu.add)
                    yg = pb.tile([P, T], bf16, name='yg', tag='yg')
                    nc.vector.scalar_tensor_tensor(
                        yg[:], xi_c[db][:, tsl], d_sb[:, db:db + 1],
                        h[:, 0:T], op0=Alu.mult, op1=Alu.add)
                    nc.vector.tensor_tensor(yg[:], yg[:], sz[db][:, tsl],
                                            op=Alu.mult)
                    if dbg and ci == 0 and db == 0:
                        ygf = pb.tile([P, T], f32, name='ygf', tag='ygf')
                        nc.vector.tensor_copy(ygf[:], yg[:])
                        nc.sync.dma_start(dbg_yg[:], ygf[:])
                    for mb in range(MB):
                        nc.tensor.matmul(
                            ps_o[mb][:], outw_sb[db][:, ts(mb, P)], yg[:],
                            start=(db == 0), stop=(db == NB - 1))
                    if db == NB - 1:
                        hf_, off = divmod(ci * T, TAIL)
                        for mb in range(MB):
                            xm_sb = pb.tile([P, T], f32, name='xm', tag='xm')
                            nc.scalar.activation(xm_sb[:], ps_o[mb][:], AFT.Copy)
                            nc.sync.dma_start(
                                xm_bo[hf_, ts(mb, P), ds(off, T)], xm_sb[:])

                pend = None
                for ci in range(NCH):
                    tsl = ds(ci * T, T)
                    # dbc AllReduce landing: load + bf16 convert + broadcast
                    # staging, per chunk (keeps DVE queue free of later-chunk
                    # AR waits -- head-of-line fix)
                    dbc_f = pb0.tile([DBC, T], f32, name='dbcf', tag='dbcf')
                    nc.sync.dma_start(dbc_f[:], dbc_ar[ci])
                    nc.vector.tensor_copy(dbc16[:, tsl], dbc_f[:])
                    nc.sync.dma_start(dbcBC[ci], dbc16[DT_RANK:DBC, tsl])
                    b_rep = pbr.tile([P, NT], bf16, name='b_rep', tag='b_rep')
                    c_rep = pbr.tile([P, NT], bf16, name='c_rep', tag='c_rep')
                    bc_flat = dbcBC[ci].rearrange('n t -> (n t)').unsqueeze(0)
                    nc.sync.dma_start(
                        b_rep[:], bc_flat[:, 0:NT].broadcast_to([P, NT]))
                    nc.sync.dma_start(
                        c_rep[:], bc_flat[:, NT:2 * NT].broadcast_to([P, NT]))
                    ps_o = [po_ps.tile([P, T], f32, name=f'pso{mb}',
                                       tag=f'pso{mb}') for mb in range(MB)]
                    for db in range(NB):
                        ps_d = pb_ps.tile([P, T], f32, name='dt', tag='dt')
                        nc.tensor.matmul(ps_d[:], dtw_sb[:, ts(db, P)],
                                         dbc16[0:DT_RANK, tsl],
                                         start=True, stop=True)
                        delta = pb.tile([P, T], f32, name='delta', tag='delta')
                        nc.scalar.activation(delta[:], ps_d[:], AFT.Exp,
                                             bias=dtb_sb[:, db:db + 1], scale=1.0)
                        nc.scalar.activation(delta[:], delta[:], AFT.Ln,
                                             bias=1.0, scale=1.0)
                        d16 = pb.tile([P, T], bf16, name='d16', tag='d16')
                        nc.vector.tensor_copy(d16[:], delta[:])
                        dA = pb.tile([P, NT], bf16, name='dA', tag='dA')
                        for n in range(NSTATE):
                            nc.scalar.activation(
                                dA[:, ts(n, T)], d16[:], AFT.Exp,
                                bias=0.0,
                                scale=asc_sb[:, db * NSTATE + n:db * NSTATE + n + 1])
                        if dbg and ci == 0 and db == 0:
                            nc.sync.dma_start(dbg_delta[:], delta[:])
                        du = pb.tile([P, T], bf16, name='du', tag='du')
                        nc.vector.tensor_tensor(du[:], d16[:], xi_c[db][:, tsl],
                                                op=Alu.mult)
                        # merged dBu: du broadcast over states via stride-0 dim
                        h = pbh.tile([P, NT], bf16, name='h', tag='h')
                        nc.vector.tensor_tensor(
                            h[:].rearrange('p (n t) -> p n t', n=NSTATE),
                            du[:].unsqueeze(1).broadcast_to([P, NSTATE, T]),
                            b_rep[:].rearrange('p (n t) -> p n t', n=NSTATE),
                            op=Alu.mult)
                        for n in range(NSTATE):
                            nc.vector.tensor_tensor_scan(
                                h[:, ts(n, T)], dA[:, ts(n, T)], h[:, ts(n, T)],
                                0.0 if ci == 0 else state[db][:, n:n + 1],
                                op0=Alu.mult, op1=Alu.add)
                        nc.vector.tensor_copy(
                            state[db][:],
                            h.rearrange('p (n t) -> p n t', n=NSTATE)[:, :, T - 1])
                        if dbg and ci == 0 and db == 0:
                            hf = pb.tile([P, NSTATE * T], f32, name='hf', tag='hf')
                            nc.vector.tensor_copy(hf[:], h[:])
                            nc.sync.dma_start(dbg_h[:], hf[:])
                        # y = sum_n C_n * h_n: mul on gpsimd NOW; the DVE-side
                        # consume (tree/gate/out_proj) is deferred one
                        # iteration so the gpsimd mul hides under the next
                        # iteration's scans
                        nc.gpsimd.tensor_tensor(h[:], h[:], c_rep[:], op=Alu.mult)
                        if pend is not None:
                            consume(*pend)
                        pend = (h, ci, db, tsl, ps_o)
                consume(*pend)

            # ---------------- Phase C: collectives ----------------
            if dbg:
                nc.sync.dma_start(dbg_xm[:, 0:TAIL], xm_bo[0])
                nc.sync.dma_start(dbg_xm[:, TAIL:], xm_bo[1])
            nc.gpsimd.collective_compute(
                'ReduceScatter', Alu.add, replica_groups=PAIRS,
                ins=[xm_bo.opt()], outs=[rs_out.opt()])
            nc.gpsimd.collective_compute(
                'AllGather', Alu.bypass, replica_groups=CROSS,
                ins=[rs_out.opt()], outs=[ag_out.opt()])
            if dbg:
                nc.sync.dma_start(dbg_rs[:], rs_out[:])
                nc.sync.dma_start(dbg_ag[0:D_MODEL, :], ag_out[0])
                nc.sync.dma_start(dbg_ag[D_MODEL:, :], ag_out[1])

        # ---------------- Phase D: tail (slabs of <=512 tokens) ----------------
        TT = min(512, TAIL)
        NTQ = TAIL // TT
        with tc.tile_pool(name='pt', bufs=1) as pt, \
             tc.tile_pool(name='ptw', bufs=1) as ptw, \
             tc.tile_pool(name='pt_ps', bufs=2, space='PSUM') as pt_ps, \
             tc.tile_pool(name='ps_st', bufs=2, space='PSUM') as ps_st:
            w11 = [pt.tile([P, D_FF], bf16, name=f'w11_{k}') for k in range(MB)]
            for k in range(MB):
                nc.sync.dma_start(w11[k][:], f1w1[ts(k, P), :])
            w12 = [pt.tile([P, D_MODEL], bf16, name=f'w12_{k}') for k in range(FB)]
            for k in range(FB):
                nc.sync.dma_start(w12[k][:], f1w2[ts(k, P), :])
            w21 = [pt.tile([P, D_FF], bf16, name=f'w21_{k}') for k in range(MB)]
            for k in range(MB):
                nc.sync.dma_start(w21[k][:], f2w1[ts(k, P), :])
            w22 = [pt.tile([P, D_MODEL], bf16, name=f'w22_{k}') for k in range(FB)]
            for k in range(FB):
                nc.sync.dma_start(w22[k][:], f2w2[ts(k, P), :])
            b11_sb = pt.tile([P, FB], f32)
            nc.sync.dma_start(b11_sb[:], f1b1[:])
            b12_sb = pt.tile([P, MB], f32)
            nc.sync.dma_start(b12_sb[:], f1b2[:])
            b21_sb = pt.tile([P, FB], f32)
            nc.sync.dma_start(b21_sb[:], f2b1[:])
            b22_sb = pt.tile([P, MB], f32)
            nc.sync.dma_start(b22_sb[:], f2b2[:])
            ln_sb = pt.tile([P, 8 * MB], f32)
            nc.sync.dma_start(ln_sb[:], lnp[:])
            ones_sb = pt.tile([P, P], bf16)
            nc.vector.memset(ones_sb[:], 1.0)
            eps_sb = pt.tile([P, 1], f32)
            nc.vector.memset(eps_sb[:], LN_EPS)

            def layer_norm(src, lni, name):
                # stats summed over features AND broadcast to all partitions
                # via all-ones [128,128] matmuls (keeps gpsimd free for AG)
                ps_s = ps_st.tile([P, TT], f32, name=f'{name}_s1', tag='stat1')
                for k in range(MB):
                    nc.tensor.matmul(ps_s[:], ones_sb[:], src[k][:],
                                     start=(k == 0), stop=(k == MB - 1))
                ps_q = ps_st.tile([P, TT], f32, name=f'{name}_s2', tag='stat2')
                sqs = []
                for k in range(MB):
                    sq = ptw.tile([P, TT], bf16, name=f'{name}_sq{k}', tag=f'sq{k}')
                    nc.scalar.activation(sq[:], src[k][:], AFT.Square)
                    sqs.append(sq)
                for k in range(MB):
                    nc.tensor.matmul(ps_q[:], ones_sb[:], sqs[k][:],
                                     start=(k == 0), stop=(k == MB - 1))
                mu = ptw.tile([P, TT], f32, name=f'{name}_mu', tag='mu')
                nc.vector.tensor_scalar(mu[:], ps_s[:], 1.0 / D_MODEL, None,
                                        op0=Alu.mult)
                var = ptw.tile([P, TT], f32, name=f'{name}_var', tag='var')
                nc.vector.tensor_tensor(var[:], mu[:], mu[:], op=Alu.mult)
                nc.vector.scalar_tensor_tensor(
                    var[:], ps_q[:], 1.0 / D_MODEL, var[:],
                    op0=Alu.mult, op1=Alu.subtract)
                # rstd = exp(-0.5*ln(var+eps)) -- two ACT LUT ops instead of
                # the slow DVE iterative reciprocal
                lv = ptw.tile([P, TT], f32, name=f'{name}_lv', tag='lv')
                nc.scalar.activation(lv[:], var[:], AFT.Ln,
                                     bias=eps_sb[:], scale=1.0)
                rstd = ptw.tile([P, TT], bf16, name=f'{name}_rstd', tag='rstd')
                nc.scalar.activation(rstd[:], lv[:], AFT.Exp,
                                     bias=0.0, scale=-0.5)
                mu16 = ptw.tile([P, TT], bf16, name=f'{name}_mu16', tag='mu16')
                nc.vector.tensor_copy(mu16[:], mu[:])
                outs = []
                for k in range(MB):
                    o = ptw.tile([P, TT], bf16, name=f'{name}_o{k}',
                                 tag=f'{name}_o{k}')
                    nc.vector.tensor_tensor(o[:], src[k][:], mu16[:],
                                            op=Alu.subtract)
                    nc.vector.tensor_tensor(o[:], o[:], rstd[:], op=Alu.mult)
                    nc.vector.tensor_scalar(
                        o[:], o[:],
                        ln_sb[:, (2 * lni) * MB + k:(2 * lni) * MB + k + 1],
                        ln_sb[:, (2 * lni + 1) * MB + k:(2 * lni + 1) * MB + k + 1],
                        op0=Alu.mult, op1=Alu.add)
                    outs.append(o)
                return outs

            def ffn(src, w1l, b1t, w2l, b2t, name):
                f1 = []
                for fb in range(FB):
                    ps_f = pt_ps.tile([P, TT], f32, name=f'{name}_f{fb}', tag='ffp')
                    for kb in range(MB):
                        nc.tensor.matmul(ps_f[:], w1l[kb][:, ts(fb, P)], src[kb][:],
                                         start=(kb == 0), stop=(kb == MB - 1))
                    r = ptw.tile([P, TT], bf16, name=f'{name}_r{fb}', tag=f'ffr{fb}')
                    nc.scalar.activation(r[:], ps_f[:], AFT.Relu,
                                         bias=b1t[:, fb:fb + 1], scale=1.0)
                    f1.append(r)
                outs = []
                for mb in range(MB):
                    ps_g = pt_ps.tile([P, TT], f32, name=f'{name}_g{mb}', tag='ffq')
                    for kb in range(FB):
                        nc.tensor.matmul(ps_g[:], w2l[kb][:, ts(mb, P)], f1[kb][:],
                                         start=(kb == 0), stop=(kb == FB - 1))
                    o = ptw.tile([P, TT], f32, name=f'{name}_o{mb}', tag=f'ffo{mb}')
                    nc.vector.tensor_scalar(o[:], ps_g[:], 1.0, b2t[:, mb:mb + 1],
                                            op0=Alu.mult, op1=Alu.add)
                    outs.append(o)
                return outs

            def dump(nm, tiles, tqs):
                if not dbg:
                    return
                for k in range(MB):
                    tt_ = ptw.tile([P, TT], f32, name=f'dmp_{nm}_{k}', tag='dmp')
                    nc.vector.tensor_copy(tt_[:], tiles[k][:])
                    nc.sync.dma_start(dbg_tails[nm][ts(k, P), tqs], tt_[:])

            for tq in range(NTQ):
                tqs = ds(tq * TT, TT)
                # x_f branch reads own rs_out (valid on f-cores; b-cores'
                # tail result is discarded by host) -> overlaps the AllGather
                r1 = []
                for k in range(MB):
                    txf = ptw.tile([P, TT], f32, name=f'txf{k}', tag=f'txf{k}')
                    nc.sync.dma_start(txf[:], tail_x[ts(k, P), tqs])
                    xf = ptw.tile([P, TT], f32, name=f'xf{k}', tag=f'xf{k}')
                    nc.sync.dma_start(xf[:], rs_out[ts(k, P), tqs])
                    a = ptw.tile([P, TT], bf16, name=f'r1_{k}', tag=f'r1_{k}')
                    nc.vector.tensor_tensor(a[:], xf[:], txf[:], op=Alu.add)
                    r1.append(a)
                dump('r1', r1, tqs)
                t1 = layer_norm(r1, 0, 'ln1')
                dump('t1', t1, tqs)
                ff1 = ffn(t1, w11, b11_sb, w12, b12_sb, 'ffn1')
                dump('ff1', ff1, tqs)
                s2 = []
                for k in range(MB):
                    s_ = ptw.tile([P, TT], bf16, name=f's2_{k}', tag=f's2_{k}')
                    nc.vector.tensor_tensor(s_[:], ff1[k][:], t1[k][:], op=Alu.add)
                    s2.append(s_)
                t2 = layer_norm(s2, 1, 'ln2')
                dump('t2', t2, tqs)
                ff2 = ffn(t2, w21, b21_sb, w22, b22_sb, 'ffn2')
                dump('ff2', ff2, tqs)
                # x_b branch: ag_out[1] holds partner's half in reversed token
                # order; un-flip on-chip with a reversed-AP DVE copy
                r3 = []
                for k in range(MB):
                    txf2 = ptw.tile([P, TT], f32, name=f'txg{k}', tag=f'txf{k}')
                    nc.sync.dma_start(txf2[:], tail_x[ts(k, P), tqs])
                    xbr = ptw.tile([P, TT], f32, name=f'xbr{k}', tag=f'xbr{k}')
                    nc.sync.dma_start(
                        xbr[:], ag_out[1, ts(k, P), ds(TAIL - (tq + 1) * TT, TT)])
                    xb = ptw.tile([P, TT], f32, name=f'xb{k}', tag=f'xb{k}')
                    nc.vector.tensor_copy(xb[:], xbr[:, ::-1])
                    bt = ptw.tile([P, TT], bf16, name=f'r3_{k}', tag=f'r3_{k}')
                    nc.vector.tensor_tensor(bt[:], xb[:], txf2[:], op=Alu.add)
                    r3.append(bt)
                dump('r3', r3, tqs)
                t3 = layer_norm(r3, 2, 'ln3')
                dump('t3', t3, tqs)
                s4 = []
                for k in range(MB):
                    s_ = ptw.tile([P, TT], bf16, name=f's4_{k}', tag=f's4_{k}')
                    nc.vector.tensor_tensor(s_[:], ff2[k][:], t3[k][:], op=Alu.add)
                    s4.append(s_)
                t4 = layer_norm(s4, 3, 'ln4')
                dump('t4', t4, tqs)
                for k in range(MB):
                    o = ptw.tile([P, TT], f32, name=f'fin{k}', tag=f'fin{k}')
                    nc.vector.tensor_tensor(o[:], t2[k][:], t4[k][:], op=Alu.add)
                    nc.sync.dma_start(out_t[ts(k, P), tqs], o[:])

        dram_cm.__exit__(None, None, None)

    nc.compile()
    return nc


def _prep_inputs(inputs, L):
    """Build per-core in_maps from the full problem inputs."""
    TAIL = L // 2
    x = np.asarray(inputs['x'])
    in_maps = []
    for c in range(NCORES):
        b, rem = divmod(c, 4)
        dire, dh = divmod(rem, 2)
        p = 'f' if dire == 0 else 'b'
        dsl = slice(dh * DH, (dh + 1) * DH)
        xs = x[b] if dire == 0 else x[b][::-1]
        m = {}
        m['xT16'] = np.ascontiguousarray(xs.T).astype(ml_dtypes.bfloat16)
        in_w = np.asarray(inputs[p + '_in_w'])
        w_xz = np.concatenate([in_w[dsl], in_w[D_INNER:][dsl]], axis=0)  # [1024,512]
        m['in_wT'] = np.ascontiguousarray(w_xz.T).astype(ml_dtypes.bfloat16)
        cw = np.asarray(inputs[p + '_conv_w'])[dsl, 0, :]                # [512,4]
        m['conv_w'] = np.ascontiguousarray(
            cw.reshape(NB, P, D_CONV).transpose(1, 0, 2).reshape(P, NB * D_CONV)
        ).astype(np.float32)
        m['conv_b'] = np.ascontiguousarray(
            np.asarray(inputs[p + '_conv_b'])[dsl].reshape(NB, P).T
        ).astype(np.float32)
        xp = np.asarray(inputs[p + '_xproj_w'])[:, dsl]                  # [64,512]
        m['xproj_wT'] = np.ascontiguousarray(xp.T).astype(ml_dtypes.bfloat16)
        dtw = np.asarray(inputs[p + '_dt_w'])[dsl]                       # [512,32]
        m['dt_wT'] = np.ascontiguousarray(dtw.T).astype(ml_dtypes.bfloat16)
        m['dt_b'] = np.ascontiguousarray(
            np.asarray(inputs[p + '_dt_b'])[dsl].reshape(NB, P).T
        ).astype(np.float32)
        A = -np.exp(np.asarray(inputs[p + '_A_log'])[dsl])               # [512,16]
        m['A_sc'] = np.ascontiguousarray(
            A.reshape(NB, P, NSTATE).transpose(1, 0, 2).reshape(P, NB * NSTATE)
        ).astype(np.float32)
        m['D_in'] = np.ascontiguousarray(
            np.asarray(inputs[p + '_D'])[dsl].reshape(NB, P).T
        ).astype(np.float32)
        ow = np.asarray(inputs[p + '_out_w'])[:, dsl]                    # [512,512]
        m['out_wT'] = np.ascontiguousarray(ow.T).astype(ml_dtypes.bfloat16)
        for nm, key in (('f1w1', 'ffn1_w1'), ('f1w2', 'ffn1_w2'),
                        ('f2w1', 'ffn2_w1'), ('f2w2', 'ffn2_w2')):
            w = np.asarray(inputs[key])
            m[nm] = np.ascontiguousarray(w.T).astype(ml_dtypes.bfloat16)
        for nm, key, n_el in (('f1b1', 'ffn1_b1', D_FF), ('f1b2', 'ffn1_b2', D_MODEL),
                              ('f2b1', 'ffn2_b1', D_FF), ('f2b2', 'ffn2_b2', D_MODEL)):
            v = np.asarray(inputs[key]).reshape(n_el // P, P).T
            m[nm] = np.ascontiguousarray(v).astype(np.float32)
        ln = []
        for i in (1, 2, 3, 4):
            for sfx in ('w', 'b'):
                v = np.asarray(inputs[f'ln{i}_{sfx}']).reshape(D_MODEL // P, P).T
                ln.append(v)
        m['lnp'] = np.ascontiguousarray(np.concatenate(ln, axis=1)).astype(np.float32)
        th = dh ^ dire
        m['tail_x'] = np.ascontiguousarray(
            x[b, th * TAIL:(th + 1) * TAIL].T).astype(np.float32)
        in_maps.append(m)
    return in_maps


_PROGRAM_CACHE = {}


def kernel(**inputs):
    L = np.asarray(inputs['x']).shape[1]
    T = min(512, L // 2)
    key = (L, T)
    if key not in _PROGRAM_CACHE:
        _PROGRAM_CACHE[key] = build_program(L, T)
    nc = _PROGRAM_CACHE[key]
    in_maps = _prep_inputs(inputs, L)
    trace = os.environ.get('BIMAMBA_TRACE', '0') == '1'
    if trace:
        try:
            import ntff_shim
            ntff_shim.install()
        except Exception:
            trace = False
    res = run_bass_kernel_spmd(nc, in_maps, list(range(NCORES)), trace=trace)
    if trace and res.exec_time_ns is not None:
        kernel.last_exec_time_ns = res.exec_time_ns
    TAIL = L // 2
    x = np.asarray(inputs['x'])
    B = x.shape[0]
    out = np.empty((B, L, D_MODEL), np.float32)
    for b in range(B):
        out[b, 0:TAIL] = res.results[b * 4 + 0]['out'].T
        out[b, TAIL:L] = res.results[b * 4 + 1]['out'].T
    return out


kernel.last_exec_time_ns = None


# revision 23
# speedup vs baseline: 1.2307x; 1.2307x over previous
"""BiMamba encoder layer on 8 Trainium2 NeuronCores (Bass/Tile, SPMD).

Sharding: core c = (batch b, direction dir, d_inner-half dh), c = b*4 + dir*2 + dh.
All 8 cores run an IDENTICAL program; per-core behavior comes only from input
data (host pre-sliced/pre-flipped weights and activations) and collective
replica groups.

v2: all DRAM bounce buffers are FEATURE-MAJOR so every DMA moves contiguous
2KB rows (v1 used token-major transposing DMAs: 2.6M 4-byte packets, 96% DMA
time). The b-direction time-flip is done on-chip with reversed-AP DVE copies;
B/C state broadcasts use gpsimd partition_broadcast instead of DRAM broadcast
DMAs; the final [D_MODEL, TAIL] -> [TAIL, D_MODEL] transpose happens on host.

Pipeline per core (512 of 1024 d_inner channels, full L for one (b, dir)):
  A) in_proj (PE, bf16) -> causal conv (STT taps) -> silu; silu(z) saved;
     x_proj partial (PE) -> per-chunk pair AllReduce (overlapped)
  B) dt (PE) -> softplus (ACT) -> dA = exp(A*delta) (ACT) -> dBu (merged DVE
     mult w/ stride-0 broadcast) -> 16 hardware scans (DVE, in-place) ->
     y = C.h (DVE mul + tree) -> gate -> out_proj partial (PE) -> xm
     feature-major [2, 512, 1024] (l'-half major)
  C) pair ReduceScatter (l' halves) -> cross AllGather
  D) tail: x_f branch (r1/LN1/FFN1/LN2/FFN2) reads own rs_out (overlaps AG);
     x_b branch (LN3/LN4/sum) reads ag_out[1] reversed on-chip. LN stats are
     broadcast via ones-matmul on PE (no gpsimd -> no queue conflict with AG).

Host: out[b, th half] = f-direction cores' [D_MODEL, TAIL] output, transposed.
"""
import sys
import os

sys.path.insert(0, '/opt/trn_rl_repo')

import numpy as np
import ml_dtypes

import concourse.bass as bass
import concourse.mybir as mybir
import concourse.tile as tile
from concourse import bacc
from concourse.bass_utils import run_bass_kernel_spmd
from concourse.bass import ds, ts

f32 = mybir.dt.float32
bf16 = mybir.dt.bfloat16
Alu = mybir.AluOpType
AFT = mybir.ActivationFunctionType

P = 128
D_MODEL = 512
D_INNER = 1024
DH = 512            # d_inner channels per core
NB = DH // P        # 4 channel blocks per core
NSTATE = 16
DT_RANK = 32
DBC = DT_RANK + 2 * NSTATE   # 64
D_CONV = 4
D_FF = 1024
NCORES = 8
LN_EPS = 1e-5

PAIRS = [[0, 1], [2, 3], [4, 5], [6, 7]]
CROSS = [[0, 3], [1, 2], [4, 7], [5, 6]]


def build_program(L, T, dbg=False):
    """Emit the SPMD program for sequence length L, phase chunk T."""
    NCH = L // T
    TAIL = L // 2
    MB = D_MODEL // P   # 4 blocks of d_model
    FB = D_FF // P      # 8 blocks of d_ff
    NT = NSTATE * T

    nc = bacc.Bacc('TRN2', target_bir_lowering=False, debug=False,
                   num_devices=NCORES)

    def din(name, shape, dt=f32):
        return nc.dram_tensor(name, shape, dt, kind='ExternalInput')

    xT16 = din('xT16', [D_MODEL, L], bf16)
    in_wT = din('in_wT', [D_MODEL, 2 * DH], bf16)       # K x (xi|z)
    conv_w = din('conv_w', [P, NB * D_CONV])
    conv_b = din('conv_b', [P, NB])
    xproj_wT = din('xproj_wT', [DH, DBC], bf16)
    dt_wT = din('dt_wT', [DT_RANK, DH], bf16)
    dt_b = din('dt_b', [P, NB])
    A_sc = din('A_sc', [P, NB * NSTATE])
    D_in = din('D_in', [P, NB])
    out_wT = din('out_wT', [DH, D_MODEL], bf16)
    f1w1 = din('f1w1', [D_MODEL, D_FF], bf16)
    f1b1 = din('f1b1', [P, D_FF // P])
    f1w2 = din('f1w2', [D_FF, D_MODEL], bf16)
    f1b2 = din('f1b2', [P, D_MODEL // P])
    f2w1 = din('f2w1', [D_MODEL, D_FF], bf16)
    f2b1 = din('f2b1', [P, D_FF // P])
    f2w2 = din('f2w2', [D_FF, D_MODEL], bf16)
    f2b2 = din('f2b2', [P, D_MODEL // P])
    lnp = din('lnp', [P, 8 * (D_MODEL // P)])           # ln1..4 w,b
    tail_x = din('tail_x', [D_MODEL, TAIL])
    out_t = nc.dram_tensor('out', [D_MODEL, TAIL], f32, kind='ExternalOutput')
    if dbg:
        dbg_xic = nc.dram_tensor('dbg_xic', [DH, L], f32, kind='ExternalOutput')
        dbg_sz = nc.dram_tensor('dbg_sz', [DH, L], f32, kind='ExternalOutput')
        dbg_dbc = nc.dram_tensor('dbg_dbc', [DBC, L], f32, kind='ExternalOutput')
        dbg_delta = nc.dram_tensor('dbg_delta', [P, T], f32, kind='ExternalOutput')
        dbg_h = nc.dram_tensor('dbg_h', [P, NSTATE * T], f32, kind='ExternalOutput')
        dbg_yg = nc.dram_tensor('dbg_yg', [P, T], f32, kind='ExternalOutput')
        dbg_xm = nc.dram_tensor('dbg_xm', [D_MODEL, L], f32, kind='ExternalOutput')
        dbg_rs = nc.dram_tensor('dbg_rs', [D_MODEL, TAIL], f32, kind='ExternalOutput')
        dbg_ag = nc.dram_tensor('dbg_ag', [2 * D_MODEL, TAIL], f32,
                                kind='ExternalOutput')
        dbg_tails = {_n: nc.dram_tensor('dbg_' + _n, [D_MODEL, TAIL], f32,
                                        kind='ExternalOutput')
                     for _n in ('r1', 't1', 'ff1', 't2', 'r3', 't3', 'ff2', 't4')}

    with tile.TileContext(nc) as tc:
        dram_cm = tc.tile_pool(name='dram', bufs=1, space='DRAM')
        dram = dram_cm.__enter__()
        dbc_bo = dram.tile([NCH, DBC, T], f32)
        dbc_ar = dram.tile([NCH, DBC, T], f32)
        dbcBC = dram.tile([NCH, 2 * NSTATE, T], bf16)
        xm_bo = dram.tile([2, D_MODEL, TAIL], f32)
        rs_out = dram.tile([D_MODEL, TAIL], f32)
        ag_out = dram.tile([2, D_MODEL, TAIL], f32)

        with tc.tile_pool(name='pers', bufs=1) as pers:
            # persistent SBUF (live through phases A-B; small ones to D)
            xpw_sb = [pers.tile([P, DBC], bf16, name=f'xpw{k}')
                      for k in range(NB)]
            for k in range(NB):
                nc.sync.dma_start(xpw_sb[k][:], xproj_wT[ts(k, P), :])
            dtw_sb = pers.tile([DT_RANK, DH], bf16)
            nc.sync.dma_start(dtw_sb[:], dt_wT[:])
            outw_sb = [pers.tile([P, D_MODEL], bf16, name=f'outw{k}') for k in range(NB)]
            for k in range(NB):
                nc.sync.dma_start(outw_sb[k][:], out_wT[ts(k, P), :])
            cw_sb = pers.tile([P, NB * D_CONV], f32)
            nc.sync.dma_start(cw_sb[:], conv_w[:])
            cb_sb = pers.tile([P, NB], f32)
            nc.sync.dma_start(cb_sb[:], conv_b[:])
            dtb_sb = pers.tile([P, NB], f32)
            nc.sync.dma_start(dtb_sb[:], dt_b[:])
            asc_sb = pers.tile([P, NB * NSTATE], f32)
            nc.sync.dma_start(asc_sb[:], A_sc[:])
            d_sb = pers.tile([P, NB], f32)
            nc.sync.dma_start(d_sb[:], D_in[:])

            xi_c = [pers.tile([P, L], bf16, name=f'xic{k}') for k in range(NB)]
            sz = [pers.tile([P, L], bf16, name=f'sz{k}') for k in range(NB)]
            halo = [pers.tile([P, D_CONV - 1], bf16, name=f'halo{k}') for k in range(NB)]
            for k in range(NB):
                nc.vector.memset(halo[k][:], 0.0)
            dbc16 = pers.tile([DBC, L], bf16)

            # ---------------- Phase A ----------------
            with tc.tile_pool(name='pax', bufs=1) as pax, \
                 tc.tile_pool(name='pa', bufs=3) as pa, \
                 tc.tile_pool(name='pa_ps', bufs=4, space='PSUM') as pa_ps:
                x_sb = [pax.tile([P, L], bf16, name=f'x_sb{k}') for k in range(MB)]
                for k in range(MB):
                    nc.sync.dma_start(x_sb[k][:], xT16[ts(k, P), :])
                inw_sb = [pax.tile([P, 2 * DH], bf16, name=f'inw{k}')
                          for k in range(MB)]
                for k in range(MB):
                    nc.sync.dma_start(inw_sb[k][:], in_wT[ts(k, P), :])
                for ci in range(NCH):
                    tsl = ds(ci * T, T)
                    for mb in range(2 * NB):   # 4 xi blocks then 4 z blocks
                        ps_t = pa_ps.tile([P, T], f32, name='inproj')
                        for kb in range(MB):
                            nc.tensor.matmul(
                                ps_t[:], inw_sb[kb][:, ts(mb, P)],
                                x_sb[kb][:, tsl],
                                start=(kb == 0), stop=(kb == MB - 1))
                        if mb < NB:
                            db = mb
                            xr = pa.tile([P, D_CONV - 1 + T], bf16, name='xr',
                                         tag='xr')
                            nc.vector.tensor_copy(xr[:, 0:D_CONV - 1], halo[db][:])
                            nc.scalar.activation(xr[:, D_CONV - 1:], ps_t[:],
                                                 AFT.Copy)
                            nc.vector.tensor_copy(halo[db][:], xr[:, T:])
                            # causal conv: tap 0 with bias, then taps 1..3
                            xc = pa.tile([P, T], bf16, name='xc', tag='xc')
                            nc.vector.tensor_scalar(
                                xc[:], xr[:, 0:T],
                                cw_sb[:, db * D_CONV:db * D_CONV + 1],
                                cb_sb[:, db:db + 1],
                                op0=Alu.mult, op1=Alu.add)
                            for k in range(1, D_CONV):
                                nc.vector.scalar_tensor_tensor(
                                    xc[:], xr[:, k:k + T],
                                    cw_sb[:, db * D_CONV + k:db * D_CONV + k + 1],
                                    xc[:], op0=Alu.mult, op1=Alu.add)
                            nc.scalar.activation(xi_c[db][:, tsl], xc[:], AFT.Silu)
                        else:
                            db = mb - NB
                            nc.scalar.activation(sz[db][:, tsl], ps_t[:], AFT.Silu)
                    # x_proj partial for this chunk -> per-chunk AllReduce
                    ps_x = pa_ps.tile([P, T], f32, name='xproj')
                    for kb in range(NB):
                        nc.tensor.matmul(
                            ps_x[0:DBC, :], xpw_sb[kb][:],
                            xi_c[kb][:, tsl],
                            start=(kb == 0), stop=(kb == NB - 1))
                    dbc_p = pa.tile([DBC, T], f32, name='dbcp', tag='dbcp')
                    nc.scalar.activation(dbc_p[:], ps_x[0:DBC, :], AFT.Copy)
                    nc.sync.dma_start(dbc_bo[ci], dbc_p[:])
                    nc.gpsimd.collective_compute(
                        'AllReduce', Alu.add, replica_groups=PAIRS,
                        ins=[dbc_bo[ci].opt()], outs=[dbc_ar[ci].opt()])



            if dbg:
                with tc.tile_pool(name='dbgp', bufs=1) as dbgp:
                    for k in range(NB):
                        t_ = dbgp.tile([P, L], f32, name='dbgt', tag='dbgt')
                        nc.vector.tensor_copy(t_[:], xi_c[k][:])
                        nc.sync.dma_start(dbg_xic[ts(k, P), :], t_[:])
                        t2_ = dbgp.tile([P, L], f32, name='dbgt2', tag='dbgt')
                        nc.vector.tensor_copy(t2_[:], sz[k][:])
                        nc.sync.dma_start(dbg_sz[ts(k, P), :], t2_[:])
                    t3_ = dbgp.tile([DBC, L], f32, name='dbgt3', tag='dbgt3')
                    nc.vector.tensor_copy(t3_[:], dbc16[:])
                    nc.sync.dma_start(dbg_dbc[:], t3_[:])

            # ---------------- Phase B ----------------
            state = [pers.tile([P, NSTATE], f32, name=f'st{k}') for k in range(NB)]
            with tc.tile_pool(name='pb', bufs=2) as pb, \
                 tc.tile_pool(name='pb0', bufs=2) as pb0, \
                 tc.tile_pool(name='pbh', bufs=2) as pbh, \
                 tc.tile_pool(name='pbr', bufs=2) as pbr, \
                 tc.tile_pool(name='pb_ps', bufs=2, space='PSUM') as pb_ps, \
                 tc.tile_pool(name='po_ps', bufs=1, space='PSUM') as po_ps:
                def consume(h, ci, db, tsl, ps_o):
                    half = NSTATE
                    while half > 1:
                        half //= 2
                        nc.vector.tensor_tensor(
                            h[:, 0:half * T], h[:, 0:half * T],
                            h[:, half * T:2 * half * T], op=Alu.add)
                    yg = pb.tile([P, T], bf16, name='yg', tag='yg')
                    nc.vector.scalar_tensor_tensor(
                        yg[:], xi_c[db][:, tsl], d_sb[:, db:db + 1],
                        h[:, 0:T], op0=Alu.mult, op1=Alu.add)
                    nc.vector.tensor_tensor(yg[:], yg[:], sz[db][:, tsl],
                                            op=Alu.mult)
                    if dbg and ci == 0 and db == 0:
                        ygf = pb.tile([P, T], f32, name='ygf', tag='ygf')
                        nc.vector.tensor_copy(ygf[:], yg[:])
                        nc.sync.dma_start(dbg_yg[:], ygf[:])
                    for mb in range(MB):
                        nc.tensor.matmul(
                            ps_o[mb][:], outw_sb[db][:, ts(mb, P)], yg[:],
                            start=(db == 0), stop=(db == NB - 1))
                    if db == NB - 1:
                        hf_, off = divmod(ci * T, TAIL)
                        for mb in range(MB):
                            xm_sb = pb.tile([P, T], f32, name='xm', tag='xm')
                            nc.scalar.activation(xm_sb[:], ps_o[mb][:], AFT.Copy)
                            nc.sync.dma_start(
                                xm_bo[hf_, ts(mb, P), ds(off, T)], xm_sb[:])

                for ci in range(NCH):
                    tsl = ds(ci * T, T)
                    # dbc AllReduce landing: load + bf16 convert + broadcast
                    # staging, per chunk (keeps DVE queue free of later-chunk
                    # AR waits -- head-of-line fix)
                    dbc_f = pb0.tile([DBC, T], f32, name='dbcf', tag='dbcf')
                    nc.sync.dma_start(dbc_f[:], dbc_ar[ci])
                    nc.vector.tensor_copy(dbc16[:, tsl], dbc_f[:])
                    nc.sync.dma_start(dbcBC[ci], dbc16[DT_RANK:DBC, tsl])
                    b_rep = pbr.tile([P, NT], bf16, name='b_rep', tag='b_rep')
                    c_rep = pbr.tile([P, NT], bf16, name='c_rep', tag='c_rep')
                    bc_flat = dbcBC[ci].rearrange('n t -> (n t)').unsqueeze(0)
                    nc.sync.dma_start(
                        b_rep[:], bc_flat[:, 0:NT].broadcast_to([P, NT]))
                    nc.sync.dma_start(
                        c_rep[:], bc_flat[:, NT:2 * NT].broadcast_to([P, NT]))
                    ps_o = [po_ps.tile([P, T], f32, name=f'pso{mb}',
                                       tag=f'pso{mb}') for mb in range(MB)]
                    for db in range(NB):
                        ps_d = pb_ps.tile([P, T], f32, name='dt', tag='dt')
                        nc.tensor.matmul(ps_d[:], dtw_sb[:, ts(db, P)],
                                         dbc16[0:DT_RANK, tsl],
                                         start=True, stop=True)
                        delta = pb.tile([P, T], f32, name='delta', tag='delta')
                        nc.scalar.activation(delta[:], ps_d[:], AFT.Exp,
                                             bias=dtb_sb[:, db:db + 1], scale=1.0)
                        nc.scalar.activation(delta[:], delta[:], AFT.Ln,
                                             bias=1.0, scale=1.0)
                        d16 = pb.tile([P, T], bf16, name='d16', tag='d16')
                        nc.vector.tensor_copy(d16[:], delta[:])
                        dA = pb.tile([P, NT], bf16, name='dA', tag='dA')
                        for n in range(NSTATE):
                            nc.scalar.activation(
                                dA[:, ts(n, T)], d16[:], AFT.Exp,
                                bias=0.0,
                                scale=asc_sb[:, db * NSTATE + n:db * NSTATE + n + 1])
                        if dbg and ci == 0 and db == 0:
                            nc.sync.dma_start(dbg_delta[:], delta[:])
                        du = pb.tile([P, T], bf16, name='du', tag='du')
                        nc.vector.tensor_tensor(du[:], d16[:], xi_c[db][:, tsl],
                                                op=Alu.mult)
                        # merged dBu: du broadcast over states via stride-0 dim
                        h = pbh.tile([P, NT], bf16, name='h', tag='h')
                        nc.vector.tensor_tensor(
                            h[:].rearrange('p (n t) -> p n t', n=NSTATE),
                            du[:].unsqueeze(1).broadcast_to([P, NSTATE, T]),
                            b_rep[:].rearrange('p (n t) -> p n t', n=NSTATE),
                            op=Alu.mult)
                        for n in range(NSTATE):
                            nc.vector.tensor_tensor_scan(
                                h[:, ts(n, T)], dA[:, ts(n, T)], h[:, ts(n, T)],
                                0.0 if ci == 0 else state[db][:, n:n + 1],
                                op0=Alu.mult, op1=Alu.add)
                        nc.vector.tensor_copy(
                            state[db][:],
                            h.rearrange('p (n t) -> p n t', n=NSTATE)[:, :, T - 1])
                        if dbg and ci == 0 and db == 0:
                            hf = pb.tile([P, NSTATE * T], f32, name='hf', tag='hf')
                            nc.vector.tensor_copy(hf[:], h[:])
                            nc.sync.dma_start(dbg_h[:], hf[:])
                        # y = sum_n C_n * h_n (gpsimd shares the DVE SBUF port,
                        # so offloading this mul to gpsimd slows DVE more than
                        # it saves -- measured; keep everything on DVE)
                        nc.vector.tensor_tensor(h[:], h[:], c_rep[:], op=Alu.mult)
                        consume(h, ci, db, tsl, ps_o)

            # ---------------- Phase C: collectives ----------------
            if dbg:
                nc.sync.dma_start(dbg_xm[:, 0:TAIL], xm_bo[0])
                nc.sync.dma_start(dbg_xm[:, TAIL:], xm_bo[1])
            nc.gpsimd.collective_compute(
                'ReduceScatter', Alu.add, replica_groups=PAIRS,
                ins=[xm_bo.opt()], outs=[rs_out.opt()])
            nc.gpsimd.collective_compute(
                'AllGather', Alu.bypass, replica_groups=CROSS,
                ins=[rs_out.opt()], outs=[ag_out.opt()])
            if dbg:
                nc.sync.dma_start(dbg_rs[:], rs_out[:])
                nc.sync.dma_start(dbg_ag[0:D_MODEL, :], ag_out[0])
                nc.sync.dma_start(dbg_ag[D_MODEL:, :], ag_out[1])

        # ---------------- Phase D: tail (slabs of <=512 tokens) ----------------
        TT = min(512, TAIL)
        NTQ = TAIL // TT
        with tc.tile_pool(name='pt', bufs=1) as pt, \
             tc.tile_pool(name='ptw', bufs=1) as ptw, \
             tc.tile_pool(name='pt_ps', bufs=2, space='PSUM') as pt_ps, \
             tc.tile_pool(name='ps_st', bufs=2, space='PSUM') as ps_st:
            w11 = [pt.tile([P, D_FF], bf16, name=f'w11_{k}') for k in range(MB)]
            for k in range(MB):
                nc.sync.dma_start(w11[k][:], f1w1[ts(k, P), :])
            w12 = [pt.tile([P, D_MODEL], bf16, name=f'w12_{k}') for k in range(FB)]
            for k in range(FB):
                nc.sync.dma_start(w12[k][:], f1w2[ts(k, P), :])
            w21 = [pt.tile([P, D_FF], bf16, name=f'w21_{k}') for k in range(MB)]
            for k in range(MB):
                nc.sync.dma_start(w21[k][:], f2w1[ts(k, P), :])
            w22 = [pt.tile([P, D_MODEL], bf16, name=f'w22_{k}') for k in range(FB)]
            for k in range(FB):
                nc.sync.dma_start(w22[k][:], f2w2[ts(k, P), :])
            b11_sb = pt.tile([P, FB], f32)
            nc.sync.dma_start(b11_sb[:], f1b1[:])
            b12_sb = pt.tile([P, MB], f32)
            nc.sync.dma_start(b12_sb[:], f1b2[:])
            b21_sb = pt.tile([P, FB], f32)
            nc.sync.dma_start(b21_sb[:], f2b1[:])
            b22_sb = pt.tile([P, MB], f32)
            nc.sync.dma_start(b22_sb[:], f2b2[:])
            ln_sb = pt.tile([P, 8 * MB], f32)
            nc.sync.dma_start(ln_sb[:], lnp[:])
            ones_sb = pt.tile([P, P], bf16)
            nc.vector.memset(ones_sb[:], 1.0)
            eps_sb = pt.tile([P, 1], f32)
            nc.vector.memset(eps_sb[:], LN_EPS)

            def layer_norm(src, lni, name):
                # stats summed over features AND broadcast to all partitions
                # via all-ones [128,128] matmuls (keeps gpsimd free for AG)
                ps_s = ps_st.tile([P, TT], f32, name=f'{name}_s1', tag='stat1')
                for k in range(MB):
                    nc.tensor.matmul(ps_s[:], ones_sb[:], src[k][:],
                                     start=(k == 0), stop=(k == MB - 1))
                ps_q = ps_st.tile([P, TT], f32, name=f'{name}_s2', tag='stat2')
                sqs = []
                for k in range(MB):
                    sq = ptw.tile([P, TT], bf16, name=f'{name}_sq{k}', tag=f'sq{k}')
                    nc.scalar.activation(sq[:], src[k][:], AFT.Square)
                    sqs.append(sq)
                for k in range(MB):
                    nc.tensor.matmul(ps_q[:], ones_sb[:], sqs[k][:],
                                     start=(k == 0), stop=(k == MB - 1))
                mu = ptw.tile([P, TT], f32, name=f'{name}_mu', tag='mu')
                nc.vector.tensor_scalar(mu[:], ps_s[:], 1.0 / D_MODEL, None,
                                        op0=Alu.mult)
                var = ptw.tile([P, TT], f32, name=f'{name}_var', tag='var')
                nc.vector.tensor_tensor(var[:], mu[:], mu[:], op=Alu.mult)
                nc.vector.scalar_tensor_tensor(
                    var[:], ps_q[:], 1.0 / D_MODEL, var[:],
                    op0=Alu.mult, op1=Alu.subtract)
                # rstd = exp(-0.5*ln(var+eps)) -- two ACT LUT ops instead of
                # the slow DVE iterative reciprocal
                lv = ptw.tile([P, TT], f32, name=f'{name}_lv', tag='lv')
                nc.scalar.activation(lv[:], var[:], AFT.Ln,
                                     bias=eps_sb[:], scale=1.0)
                rstd = ptw.tile([P, TT], bf16, name=f'{name}_rstd', tag='rstd')
                nc.scalar.activation(rstd[:], lv[:], AFT.Exp,
                                     bias=0.0, scale=-0.5)
                mu16 = ptw.tile([P, TT], bf16, name=f'{name}_mu16', tag='mu16')
                nc.vector.tensor_copy(mu16[:], mu[:])
                outs = []
                for k in range(MB):
                    o = ptw.tile([P, TT], bf16, name=f'{name}_o{k}',
                                 tag=f'{name}_o{k}')
                    nc.vector.tensor_tensor(o[:], src[k][:], mu16[:],
                                            op=Alu.subtract)
                    nc.vector.tensor_tensor(o[:], o[:], rstd[:], op=Alu.mult)
                    nc.vector.tensor_scalar(
                        o[:], o[:],
                        ln_sb[:, (2 * lni) * MB + k:(2 * lni) * MB + k + 1],
                        ln_sb[:, (2 * lni + 1) * MB + k:(2 * lni + 1) * MB + k + 1],
                        op0=Alu.mult, op1=Alu.add)
                    outs.append(o)
                return outs

            def ffn(src, w1l, b1t, w2l, b2t, name):
                f1 = []
                for fb in range(FB):
                    ps_f = pt_ps.tile([P, TT], f32, name=f'{name}_f{fb}', tag='ffp')
                    for kb in range(MB):
                        nc.tensor.matmul(ps_f[:], w1l[kb][:, ts(fb, P)], src[kb][:],
                                         start=(kb == 0), stop=(kb == MB - 1))
                    r = ptw.tile([P, TT], bf16, name=f'{name}_r{fb}', tag=f'ffr{fb}')
                    nc.scalar.activation(r[:], ps_f[:], AFT.Relu,
                                         bias=b1t[:, fb:fb + 1], scale=1.0)
                    f1.append(r)
                outs = []
                for mb in range(MB):
                    ps_g = pt_ps.tile([P, TT], f32, name=f'{name}_g{mb}', tag='ffq')
                    for kb in range(FB):
                        nc.tensor.matmul(ps_g[:], w2l[kb][:, ts(mb, P)], f1[kb][:],
                                         start=(kb == 0), stop=(kb == FB - 1))
                    o = ptw.tile([P, TT], f32, name=f'{name}_o{mb}', tag=f'ffo{mb}')
                    nc.vector.tensor_scalar(o[:], ps_g[:], 1.0, b2t[:, mb:mb + 1],
                                            op0=Alu.mult, op1=Alu.add)
                    outs.append(o)
                return outs

            def dump(nm, tiles, tqs):
                if not dbg:
                    return
                for k in range(MB):
                    tt_ = ptw.tile([P, TT], f32, name=f'dmp_{nm}_{k}', tag='dmp')
                    nc.vector.tensor_copy(tt_[:], tiles[k][:])
                    nc.sync.dma_start(dbg_tails[nm][ts(k, P), tqs], tt_[:])

            for tq in range(NTQ):
                tqs = ds(tq * TT, TT)
                # x_f branch reads own rs_out (valid on f-cores; b-cores'
                # tail result is discarded by host) -> overlaps the AllGather
                r1 = []
                for k in range(MB):
                    txf = ptw.tile([P, TT], f32, name=f'txf{k}', tag=f'txf{k}')
                    nc.sync.dma_start(txf[:], tail_x[ts(k, P), tqs])
                    xf = ptw.tile([P, TT], f32, name=f'xf{k}', tag=f'xf{k}')
                    nc.sync.dma_start(xf[:], rs_out[ts(k, P), tqs])
                    a = ptw.tile([P, TT], bf16, name=f'r1_{k}', tag=f'r1_{k}')
                    nc.vector.tensor_tensor(a[:], xf[:], txf[:], op=Alu.add)
                    r1.append(a)
                dump('r1', r1, tqs)
                t1 = layer_norm(r1, 0, 'ln1')
                dump('t1', t1, tqs)
                ff1 = ffn(t1, w11, b11_sb, w12, b12_sb, 'ffn1')
                dump('ff1', ff1, tqs)
                s2 = []
                for k in range(MB):
                    s_ = ptw.tile([P, TT], bf16, name=f's2_{k}', tag=f's2_{k}')
                    nc.vector.tensor_tensor(s_[:], ff1[k][:], t1[k][:], op=Alu.add)
                    s2.append(s_)
                t2 = layer_norm(s2, 1, 'ln2')
                dump('t2', t2, tqs)
                ff2 = ffn(t2, w21, b21_sb, w22, b22_sb, 'ffn2')
                dump('ff2', ff2, tqs)
                # x_b branch: ag_out[1] holds partner's half in reversed token
                # order; un-flip on-chip with a reversed-AP DVE copy
                r3 = []
                for k in range(MB):
                    txf2 = ptw.tile([P, TT], f32, name=f'txg{k}', tag=f'txf{k}')
                    nc.sync.dma_start(txf2[:], tail_x[ts(k, P), tqs])
                    xbr = ptw.tile([P, TT], f32, name=f'xbr{k}', tag=f'xbr{k}')
                    nc.sync.dma_start(
                        xbr[:], ag_out[1, ts(k, P), ds(TAIL - (tq + 1) * TT, TT)])
                    xb = ptw.tile([P, TT], f32, name=f'xb{k}', tag=f'xb{k}')
                    nc.vector.tensor_copy(xb[:], xbr[:, ::-1])
                    bt = ptw.tile([P, TT], bf16, name=f'r3_{k}', tag=f'r3_{k}')
                    nc.vector.tensor_tensor(bt[:], xb[:], txf2[:], op=Alu.add)
                    r3.append(bt)
                dump('r3', r3, tqs)
                t3 = layer_norm(r3, 2, 'ln3')
                dump('t3', t3, tqs)
                s4 = []
                for k in range(MB):
                    s_ = ptw.tile([P, TT], bf16, name=f's4_{k}', tag=f's4_{k}')
                    nc.vector.tensor_tensor(s_[:], ff2[k][:], t3[k][:], op=Alu.add)
                    s4.append(s_)
                t4 = layer_norm(s4, 3, 'ln4')
                dump('t4', t4, tqs)
                for k in range(MB):
                    o = ptw.tile([P, TT], f32, name=f'fin{k}', tag=f'fin{k}')
                    nc.vector.tensor_tensor(o[:], t2[k][:], t4[k][:], op=Alu.add)
                    nc.sync.dma_start(out_t[ts(k, P), tqs], o[:])

        dram_cm.__exit__(None, None, None)

    nc.compile()
    return nc


def _prep_inputs(inputs, L):
    """Build per-core in_maps from the full problem inputs."""
    TAIL = L // 2
    x = np.asarray(inputs['x'])
    in_maps = []
    for c in range(NCORES):
        b, rem = divmod(c, 4)
        dire, dh = divmod(rem, 2)
        p = 'f' if dire == 0 else 'b'
        dsl = slice(dh * DH, (dh + 1) * DH)
        xs = x[b] if dire == 0 else x[b][::-1]
        m = {}
        m['xT16'] = np.ascontiguousarray(xs.T).astype(ml_dtypes.bfloat16)
        in_w = np.asarray(inputs[p + '_in_w'])
        w_xz = np.concatenate([in_w[dsl], in_w[D_INNER:][dsl]], axis=0)  # [1024,512]
        m['in_wT'] = np.ascontiguousarray(w_xz.T).astype(ml_dtypes.bfloat16)
        cw = np.asarray(inputs[p + '_conv_w'])[dsl, 0, :]                # [512,4]
        m['conv_w'] = np.ascontiguousarray(
            cw.reshape(NB, P, D_CONV).transpose(1, 0, 2).reshape(P, NB * D_CONV)
        ).astype(np.float32)
        m['conv_b'] = np.ascontiguousarray(
            np.asarray(inputs[p + '_conv_b'])[dsl].reshape(NB, P).T
        ).astype(np.float32)
        xp = np.asarray(inputs[p + '_xproj_w'])[:, dsl]                  # [64,512]
        m['xproj_wT'] = np.ascontiguousarray(xp.T).astype(ml_dtypes.bfloat16)
        dtw = np.asarray(inputs[p + '_dt_w'])[dsl]                       # [512,32]
        m['dt_wT'] = np.ascontiguousarray(dtw.T).astype(ml_dtypes.bfloat16)
        m['dt_b'] = np.ascontiguousarray(
            np.asarray(inputs[p + '_dt_b'])[dsl].reshape(NB, P).T
        ).astype(np.float32)
        A = -np.exp(np.asarray(inputs[p + '_A_log'])[dsl])               # [512,16]
        m['A_sc'] = np.ascontiguousarray(
            A.reshape(NB, P, NSTATE).transpose(1, 0, 2).reshape(P, NB * NSTATE)
        ).astype(np.float32)
        m['D_in'] = np.ascontiguousarray(
            np.asarray(inputs[p + '_D'])[dsl].reshape(NB, P).T
        ).astype(np.float32)
        ow = np.asarray(inputs[p + '_out_w'])[:, dsl]                    # [512,512]
        m['out_wT'] = np.ascontiguousarray(ow.T).astype(ml_dtypes.bfloat16)
        for nm, key in (('f1w1', 'ffn1_w1'), ('f1w2', 'ffn1_w2'),
                        ('f2w1', 'ffn2_w1'), ('f2w2', 'ffn2_w2')):
            w = np.asarray(inputs[key])
            m[nm] = np.ascontiguousarray(w.T).astype(ml_dtypes.bfloat16)
        for nm, key, n_el in (('f1b1', 'ffn1_b1', D_FF), ('f1b2', 'ffn1_b2', D_MODEL),
                              ('f2b1', 'ffn2_b1', D_FF), ('f2b2', 'ffn2_b2', D_MODEL)):
            v = np.asarray(inputs[key]).reshape(n_el // P, P).T
            m[nm] = np.ascontiguousarray(v).astype(np.float32)
        ln = []
        for i in (1, 2, 3, 4):
            for sfx in ('w', 'b'):
                v = np.asarray(inputs[f'ln{i}_{sfx}']).reshape(D_MODEL // P, P).T
                ln.append(v)
        m['lnp'] = np.ascontiguousarray(np.concatenate(ln, axis=1)).astype(np.float32)
        th = dh ^ dire
        m['tail_x'] = np.ascontiguousarray(
            x[b, th * TAIL:(th + 1) * TAIL].T).astype(np.float32)
        in_maps.append(m)
    return in_maps


_PROGRAM_CACHE = {}


def kernel(**inputs):
    L = np.asarray(inputs['x']).shape[1]
    T = min(512, L // 2)
    key = (L, T)
    if key not in _PROGRAM_CACHE:
        _PROGRAM_CACHE[key] = build_program(L, T)
    nc = _PROGRAM_CACHE[key]
    in_maps = _prep_inputs(inputs, L)
    trace = os.environ.get('BIMAMBA_TRACE', '0') == '1'
    if trace:
        try:
            import ntff_shim
            ntff_shim.install()
        except Exception:
            trace = False
    res = run_bass_kernel_spmd(nc, in_maps, list(range(NCORES)), trace=trace)
    if trace and res.exec_time_ns is not None:
        kernel.last_exec_time_ns = res.exec_time_ns
    TAIL = L // 2
    x = np.asarray(inputs['x'])
    B = x.shape[0]
    out = np.empty((B, L, D_MODEL), np.float32)
    for b in range(B):
        out[b, 0:TAIL] = res.results[b * 4 + 0]['out'].T
        out[b, TAIL:L] = res.results[b * 4 + 1]['out'].T
    return out


kernel.last_exec_time_ns = None


# revision 31
# speedup vs baseline: 1.3087x; 1.0634x over previous
"""BiMamba encoder layer on 8 Trainium2 NeuronCores (Bass/Tile, SPMD).

Sharding: core c = (batch b, direction dir, d_inner-half dh), c = b*4 + dir*2 + dh.
All 8 cores run an IDENTICAL program; per-core behavior comes only from input
data (host pre-sliced/pre-flipped weights and activations) and collective
replica groups.

v2: all DRAM bounce buffers are FEATURE-MAJOR so every DMA moves contiguous
2KB rows (v1 used token-major transposing DMAs: 2.6M 4-byte packets, 96% DMA
time). The b-direction time-flip is done on-chip with reversed-AP DVE copies;
B/C state broadcasts use gpsimd partition_broadcast instead of DRAM broadcast
DMAs; the final [D_MODEL, TAIL] -> [TAIL, D_MODEL] transpose happens on host.

Pipeline per core (512 of 1024 d_inner channels, full L for one (b, dir)):
  A) in_proj (PE, bf16) -> causal conv (STT taps) -> silu; silu(z) saved;
     x_proj partial (PE) -> per-chunk pair AllReduce (overlapped)
  B) dt (PE) -> softplus (ACT) -> dA = exp(A*delta) (ACT) -> dBu (merged DVE
     mult w/ stride-0 broadcast) -> 16 hardware scans (DVE, in-place) ->
     y = C.h (DVE mul + tree) -> gate -> out_proj partial (PE) -> xm
     feature-major [2, 512, 1024] (l'-half major)
  C) pair ReduceScatter (l' halves) -> cross AllGather
  D) tail: x_f branch (r1/LN1/FFN1/LN2/FFN2) reads own rs_out (overlaps AG);
     x_b branch (LN3/LN4/sum) reads ag_out[1] reversed on-chip. LN stats are
     broadcast via ones-matmul on PE (no gpsimd -> no queue conflict with AG).

Host: out[b, th half] = f-direction cores' [D_MODEL, TAIL] output, transposed.
"""
import sys
import os

sys.path.insert(0, '/opt/trn_rl_repo')

import numpy as np
import ml_dtypes

import concourse.bass as bass
import concourse.mybir as mybir
import concourse.tile as tile
from concourse import bacc
from concourse.bass_utils import run_bass_kernel_spmd
from concourse.bass import ds, ts

f32 = mybir.dt.float32
bf16 = mybir.dt.bfloat16
Alu = mybir.AluOpType
AFT = mybir.ActivationFunctionType

P = 128
D_MODEL = 512
D_INNER = 1024
DH = 512            # d_inner channels per core
NB = DH // P        # 4 channel blocks per core
NSTATE = 16
DT_RANK = 32
DBC = DT_RANK + 2 * NSTATE   # 64
D_CONV = 4
D_FF = 1024
NCORES = 8
LN_EPS = 1e-5

PAIRS = [[0, 1], [2, 3], [4, 5], [6, 7]]
CROSS = [[0, 3], [1, 2], [4, 7], [5, 6]]


def build_program(L, T, dbg=False):
    """Emit the SPMD program for sequence length L, phase chunk T."""
    NCH = L // T
    TAIL = L // 2
    MB = D_MODEL // P   # 4 blocks of d_model
    FB = D_FF // P      # 8 blocks of d_ff
    NT = NSTATE * T

    nc = bacc.Bacc('TRN2', target_bir_lowering=False, debug=False,
                   num_devices=NCORES)

    def din(name, shape, dt=f32):
        return nc.dram_tensor(name, shape, dt, kind='ExternalInput')

    xT16 = din('xT16', [D_MODEL, L], bf16)
    in_wT = din('in_wT', [D_MODEL, 2 * DH], bf16)       # K x (xi|z)
    conv_w = din('conv_w', [P, NB * D_CONV])
    conv_b = din('conv_b', [P, NB])
    xproj_wT = din('xproj_wT', [DH, DBC], bf16)
    dt_wT = din('dt_wT', [DT_RANK, DH], bf16)
    dt_b = din('dt_b', [P, NB])
    A_sc = din('A_sc', [P, NB * NSTATE])
    D_in = din('D_in', [P, NB])
    out_wT = din('out_wT', [DH, D_MODEL], bf16)
    f1w1 = din('f1w1', [D_MODEL, D_FF], bf16)
    f1b1 = din('f1b1', [P, D_FF // P])
    f1w2 = din('f1w2', [D_FF, D_MODEL], bf16)
    f1b2 = din('f1b2', [P, D_MODEL // P])
    f2w1 = din('f2w1', [D_MODEL, D_FF], bf16)
    f2b1 = din('f2b1', [P, D_FF // P])
    f2w2 = din('f2w2', [D_FF, D_MODEL], bf16)
    f2b2 = din('f2b2', [P, D_MODEL // P])
    lnp = din('lnp', [P, 8 * (D_MODEL // P)])           # ln1..4 w,b
    tail_x = din('tail_x', [D_MODEL, TAIL])
    out_t = nc.dram_tensor('out', [D_MODEL, TAIL], f32, kind='ExternalOutput')
    if dbg:
        dbg_xic = nc.dram_tensor('dbg_xic', [DH, L], f32, kind='ExternalOutput')
        dbg_sz = nc.dram_tensor('dbg_sz', [DH, L], f32, kind='ExternalOutput')
        dbg_dbc = nc.dram_tensor('dbg_dbc', [DBC, L], f32, kind='ExternalOutput')
        dbg_delta = nc.dram_tensor('dbg_delta', [P, T], f32, kind='ExternalOutput')
        dbg_h = nc.dram_tensor('dbg_h', [P, NSTATE * T], f32, kind='ExternalOutput')
        dbg_yg = nc.dram_tensor('dbg_yg', [P, T], f32, kind='ExternalOutput')
        dbg_xm = nc.dram_tensor('dbg_xm', [D_MODEL, L], bf16, kind='ExternalOutput')
        dbg_rs = nc.dram_tensor('dbg_rs', [D_MODEL, TAIL], bf16, kind='ExternalOutput')
        dbg_ag = nc.dram_tensor('dbg_ag', [2 * D_MODEL, TAIL], bf16,
                                kind='ExternalOutput')
        dbg_tails = {_n: nc.dram_tensor('dbg_' + _n, [D_MODEL, TAIL], f32,
                                        kind='ExternalOutput')
                     for _n in ('r1', 't1', 'ff1', 't2', 'r3', 't3', 'ff2', 't4')}

    with tile.TileContext(nc) as tc:
        dram_cm = tc.tile_pool(name='dram', bufs=1, space='DRAM')
        dram = dram_cm.__enter__()
        dbc_bo = dram.tile([NCH, DBC, T], f32)
        dbc_ar = dram.tile([NCH, DBC, T], f32)
        dbcBC = dram.tile([NCH, 2 * NSTATE, T], bf16)
        xm_bo = dram.tile([2, D_MODEL, TAIL], bf16)
        rs_out = dram.tile([D_MODEL, TAIL], bf16)
        ag_out = dram.tile([2, D_MODEL, TAIL], bf16)

        with tc.tile_pool(name='pers', bufs=1) as pers:
            # persistent SBUF (live through phases A-B; small ones to D)
            xpw_sb = [pers.tile([P, DBC], bf16, name=f'xpw{k}')
                      for k in range(NB)]
            for k in range(NB):
                nc.sync.dma_start(xpw_sb[k][:], xproj_wT[ts(k, P), :])
            dtw_sb = pers.tile([DT_RANK, DH], bf16)
            nc.sync.dma_start(dtw_sb[:], dt_wT[:])
            outw_sb = [pers.tile([P, D_MODEL], bf16, name=f'outw{k}') for k in range(NB)]
            for k in range(NB):
                nc.sync.dma_start(outw_sb[k][:], out_wT[ts(k, P), :])
            cw_sb = pers.tile([P, NB * D_CONV], f32)
            nc.sync.dma_start(cw_sb[:], conv_w[:])
            cb_sb = pers.tile([P, NB], f32)
            nc.sync.dma_start(cb_sb[:], conv_b[:])
            dtb_sb = pers.tile([P, NB], f32)
            nc.sync.dma_start(dtb_sb[:], dt_b[:])
            asc_sb = pers.tile([P, NB * NSTATE], f32)
            nc.sync.dma_start(asc_sb[:], A_sc[:])
            d_sb = pers.tile([P, NB], f32)
            nc.sync.dma_start(d_sb[:], D_in[:])

            xi_c = [pers.tile([P, L], bf16, name=f'xic{k}') for k in range(NB)]
            sz = [pers.tile([P, L], bf16, name=f'sz{k}') for k in range(NB)]
            halo = [pers.tile([P, D_CONV - 1], bf16, name=f'halo{k}') for k in range(NB)]
            for k in range(NB):
                nc.vector.memset(halo[k][:], 0.0)
            dbc16 = pers.tile([DBC, L], bf16)

            # ---------------- Phase A ----------------
            with tc.tile_pool(name='pax', bufs=1) as pax, \
                 tc.tile_pool(name='pa', bufs=3) as pa, \
                 tc.tile_pool(name='pa_ps', bufs=4, space='PSUM') as pa_ps:
                x_sb = [pax.tile([P, L], bf16, name=f'x_sb{k}') for k in range(MB)]
                for k in range(MB):
                    nc.sync.dma_start(x_sb[k][:], xT16[ts(k, P), :])
                inw_sb = [pax.tile([P, 2 * DH], bf16, name=f'inw{k}')
                          for k in range(MB)]
                for k in range(MB):
                    nc.sync.dma_start(inw_sb[k][:], in_wT[ts(k, P), :])
                for ci in range(NCH):
                    tsl = ds(ci * T, T)
                    for mb in range(2 * NB):   # 4 xi blocks then 4 z blocks
                        ps_t = pa_ps.tile([P, T], f32, name='inproj')
                        for kb in range(MB):
                            nc.tensor.matmul(
                                ps_t[:], inw_sb[kb][:, ts(mb, P)],
                                x_sb[kb][:, tsl],
                                start=(kb == 0), stop=(kb == MB - 1))
                        if mb < NB:
                            db = mb
                            xr = pa.tile([P, D_CONV - 1 + T], bf16, name='xr',
                                         tag='xr')
                            nc.vector.tensor_copy(xr[:, 0:D_CONV - 1], halo[db][:])
                            nc.scalar.activation(xr[:, D_CONV - 1:], ps_t[:],
                                                 AFT.Copy)
                            nc.vector.tensor_copy(halo[db][:], xr[:, T:])
                            # causal conv: tap 0 with bias, then taps 1..3
                            xc = pa.tile([P, T], bf16, name='xc', tag='xc')
                            nc.vector.tensor_scalar(
                                xc[:], xr[:, 0:T],
                                cw_sb[:, db * D_CONV:db * D_CONV + 1],
                                cb_sb[:, db:db + 1],
                                op0=Alu.mult, op1=Alu.add)
                            for k in range(1, D_CONV):
                                nc.vector.scalar_tensor_tensor(
                                    xc[:], xr[:, k:k + T],
                                    cw_sb[:, db * D_CONV + k:db * D_CONV + k + 1],
                                    xc[:], op0=Alu.mult, op1=Alu.add)
                            nc.scalar.activation(xi_c[db][:, tsl], xc[:], AFT.Silu)
                        else:
                            # raw z copied on DVE (phase A ACT is the
                            # head-of-line for phase B's delta); silu applied
                            # in the consume stage on ACT
                            db = mb - NB
                            nc.vector.tensor_copy(sz[db][:, tsl], ps_t[:])
                    # x_proj partial for this chunk -> per-chunk AllReduce
                    ps_x = pa_ps.tile([P, T], f32, name='xproj')
                    for kb in range(NB):
                        nc.tensor.matmul(
                            ps_x[0:DBC, :], xpw_sb[kb][:],
                            xi_c[kb][:, tsl],
                            start=(kb == 0), stop=(kb == NB - 1))
                    dbc_p = pa.tile([DBC, T], f32, name='dbcp', tag='dbcp')
                    nc.scalar.activation(dbc_p[:], ps_x[0:DBC, :], AFT.Copy)
                    nc.sync.dma_start(dbc_bo[ci], dbc_p[:])
                    nc.gpsimd.collective_compute(
                        'AllReduce', Alu.add, replica_groups=PAIRS,
                        ins=[dbc_bo[ci].opt()], outs=[dbc_ar[ci].opt()])



            if dbg:
                with tc.tile_pool(name='dbgp', bufs=1) as dbgp:
                    for k in range(NB):
                        t_ = dbgp.tile([P, L], f32, name='dbgt', tag='dbgt')
                        nc.vector.tensor_copy(t_[:], xi_c[k][:])
                        nc.sync.dma_start(dbg_xic[ts(k, P), :], t_[:])
                        t2_ = dbgp.tile([P, L], f32, name='dbgt2', tag='dbgt')
                        nc.vector.tensor_copy(t2_[:], sz[k][:])
                        nc.sync.dma_start(dbg_sz[ts(k, P), :], t2_[:])
                    t3_ = dbgp.tile([DBC, L], f32, name='dbgt3', tag='dbgt3')
                    nc.vector.tensor_copy(t3_[:], dbc16[:])
                    nc.sync.dma_start(dbg_dbc[:], t3_[:])

            # ---------------- Phase B ----------------
            state = [pers.tile([P, NSTATE], f32, name=f'st{k}') for k in range(NB)]
            with tc.tile_pool(name='pb', bufs=2) as pb, \
                 tc.tile_pool(name='pb0', bufs=2) as pb0, \
                 tc.tile_pool(name='pbh', bufs=2) as pbh, \
                 tc.tile_pool(name='pbr', bufs=2) as pbr, \
                 tc.tile_pool(name='pb_ps', bufs=2, space='PSUM') as pb_ps, \
                 tc.tile_pool(name='po_ps', bufs=1, space='PSUM') as po_ps:
                def consume(h, ci, db, tsl, ps_o):
                    half = NSTATE
                    while half > 1:
                        half //= 2
                        nc.vector.tensor_tensor(
                            h[:, 0:half * T], h[:, 0:half * T],
                            h[:, half * T:2 * half * T], op=Alu.add)
                    yg = pb.tile([P, T], bf16, name='yg', tag='yg')
                    nc.vector.scalar_tensor_tensor(
                        yg[:], xi_c[db][:, tsl], d_sb[:, db:db + 1],
                        h[:, 0:T], op0=Alu.mult, op1=Alu.add)
                    sgate = pb.tile([P, T], bf16, name='sg', tag='sg')
                    nc.scalar.activation(sgate[:], sz[db][:, tsl], AFT.Silu)
                    nc.vector.tensor_tensor(yg[:], yg[:], sgate[:],
                                            op=Alu.mult)
                    if dbg and ci == 0 and db == 0:
                        ygf = pb.tile([P, T], f32, name='ygf', tag='ygf')
                        nc.vector.tensor_copy(ygf[:], yg[:])
                        nc.sync.dma_start(dbg_yg[:], ygf[:])
                    for mb in range(MB):
                        nc.tensor.matmul(
                            ps_o[mb][:], outw_sb[db][:, ts(mb, P)], yg[:],
                            start=(db == 0), stop=(db == NB - 1))
                    if db == NB - 1:
                        hf_, off = divmod(ci * T, TAIL)
                        for mb in range(MB):
                            xm_sb = pb.tile([P, T], bf16, name='xm', tag='xm')
                            nc.scalar.activation(xm_sb[:], ps_o[mb][:], AFT.Copy)
                            nc.sync.dma_start(
                                xm_bo[hf_, ts(mb, P), ds(off, T)], xm_sb[:])

                for ci in range(NCH):
                    tsl = ds(ci * T, T)
                    # dbc AllReduce landing: load + bf16 convert + broadcast
                    # staging, per chunk (keeps DVE queue free of later-chunk
                    # AR waits -- head-of-line fix)
                    dbc_f = pb0.tile([DBC, T], f32, name='dbcf', tag='dbcf')
                    nc.sync.dma_start(dbc_f[:], dbc_ar[ci])
                    nc.vector.tensor_copy(dbc16[:, tsl], dbc_f[:])
                    nc.sync.dma_start(dbcBC[ci], dbc16[DT_RANK:DBC, tsl])
                    b_rep = pbr.tile([P, NT], bf16, name='b_rep', tag='b_rep')
                    c_rep = pbr.tile([P, NT], bf16, name='c_rep', tag='c_rep')
                    bc_flat = dbcBC[ci].rearrange('n t -> (n t)').unsqueeze(0)
                    nc.sync.dma_start(
                        b_rep[:], bc_flat[:, 0:NT].broadcast_to([P, NT]))
                    nc.sync.dma_start(
                        c_rep[:], bc_flat[:, NT:2 * NT].broadcast_to([P, NT]))
                    ps_o = [po_ps.tile([P, T], f32, name=f'pso{mb}',
                                       tag=f'pso{mb}') for mb in range(MB)]
                    for db in range(NB):
                        ps_d = pb_ps.tile([P, T], f32, name='dt', tag='dt')
                        nc.tensor.matmul(ps_d[:], dtw_sb[:, ts(db, P)],
                                         dbc16[0:DT_RANK, tsl],
                                         start=True, stop=True)
                        delta = pb.tile([P, T], f32, name='delta', tag='delta')
                        nc.scalar.activation(delta[:], ps_d[:], AFT.Exp,
                                             bias=dtb_sb[:, db:db + 1], scale=1.0)
                        nc.scalar.activation(delta[:], delta[:], AFT.Ln,
                                             bias=1.0, scale=1.0)
                        d16 = pb.tile([P, T], bf16, name='d16', tag='d16')
                        nc.vector.tensor_copy(d16[:], delta[:])
                        dA = pb.tile([P, NT], bf16, name='dA', tag='dA')
                        for n in range(NSTATE):
                            nc.scalar.activation(
                                dA[:, ts(n, T)], d16[:], AFT.Exp,
                                bias=0.0,
                                scale=asc_sb[:, db * NSTATE + n:db * NSTATE + n + 1])
                        if dbg and ci == 0 and db == 0:
                            nc.sync.dma_start(dbg_delta[:], delta[:])
                        du = pb.tile([P, T], bf16, name='du', tag='du')
                        nc.vector.tensor_tensor(du[:], d16[:], xi_c[db][:, tsl],
                                                op=Alu.mult)
                        # merged dBu: du broadcast over states via stride-0 dim
                        h = pbh.tile([P, NT], bf16, name='h', tag='h')
                        nc.vector.tensor_tensor(
                            h[:].rearrange('p (n t) -> p n t', n=NSTATE),
                            du[:].unsqueeze(1).broadcast_to([P, NSTATE, T]),
                            b_rep[:].rearrange('p (n t) -> p n t', n=NSTATE),
                            op=Alu.mult)
                        for n in range(NSTATE):
                            nc.vector.tensor_tensor_scan(
                                h[:, ts(n, T)], dA[:, ts(n, T)], h[:, ts(n, T)],
                                0.0 if ci == 0 else state[db][:, n:n + 1],
                                op0=Alu.mult, op1=Alu.add)
                        nc.vector.tensor_copy(
                            state[db][:],
                            h.rearrange('p (n t) -> p n t', n=NSTATE)[:, :, T - 1])
                        if dbg and ci == 0 and db == 0:
                            hf = pb.tile([P, NSTATE * T], f32, name='hf', tag='hf')
                            nc.vector.tensor_copy(hf[:], h[:])
                            nc.sync.dma_start(dbg_h[:], hf[:])
                        # y = sum_n C_n * h_n (gpsimd shares the DVE SBUF port,
                        # so offloading this mul to gpsimd slows DVE more than
                        # it saves -- measured; keep everything on DVE)
                        nc.vector.tensor_tensor(h[:], h[:], c_rep[:], op=Alu.mult)
                        consume(h, ci, db, tsl, ps_o)

            # ---------------- Phase C: collectives ----------------
            if dbg:
                nc.sync.dma_start(dbg_xm[:, 0:TAIL], xm_bo[0])
                nc.sync.dma_start(dbg_xm[:, TAIL:], xm_bo[1])
            nc.gpsimd.collective_compute(
                'ReduceScatter', Alu.add, replica_groups=PAIRS,
                ins=[xm_bo.opt()], outs=[rs_out.opt()])
            nc.gpsimd.collective_compute(
                'AllGather', Alu.bypass, replica_groups=CROSS,
                ins=[rs_out.opt()], outs=[ag_out.opt()])
            if dbg:
                nc.sync.dma_start(dbg_rs[:], rs_out[:])
                nc.sync.dma_start(dbg_ag[0:D_MODEL, :], ag_out[0])
                nc.sync.dma_start(dbg_ag[D_MODEL:, :], ag_out[1])

        # ---------------- Phase D: tail (slabs of <=512 tokens) ----------------
        TT = min(512, TAIL)
        NTQ = TAIL // TT
        with tc.tile_pool(name='pt', bufs=1) as pt, \
             tc.tile_pool(name='ptw', bufs=2) as ptw, \
             tc.tile_pool(name='pt_ps', bufs=2, space='PSUM') as pt_ps, \
             tc.tile_pool(name='ps_st', bufs=2, space='PSUM') as ps_st:
            w11 = [pt.tile([P, D_FF], bf16, name=f'w11_{k}') for k in range(MB)]
            for k in range(MB):
                nc.sync.dma_start(w11[k][:], f1w1[ts(k, P), :])
            w12 = [pt.tile([P, D_MODEL], bf16, name=f'w12_{k}') for k in range(FB)]
            for k in range(FB):
                nc.sync.dma_start(w12[k][:], f1w2[ts(k, P), :])
            w21 = [pt.tile([P, D_FF], bf16, name=f'w21_{k}') for k in range(MB)]
            for k in range(MB):
                nc.sync.dma_start(w21[k][:], f2w1[ts(k, P), :])
            w22 = [pt.tile([P, D_MODEL], bf16, name=f'w22_{k}') for k in range(FB)]
            for k in range(FB):
                nc.sync.dma_start(w22[k][:], f2w2[ts(k, P), :])
            b11_sb = pt.tile([P, FB], f32)
            nc.sync.dma_start(b11_sb[:], f1b1[:])
            b12_sb = pt.tile([P, MB], f32)
            nc.sync.dma_start(b12_sb[:], f1b2[:])
            b21_sb = pt.tile([P, FB], f32)
            nc.sync.dma_start(b21_sb[:], f2b1[:])
            b22_sb = pt.tile([P, MB], f32)
            nc.sync.dma_start(b22_sb[:], f2b2[:])
            ln_sb = pt.tile([P, 8 * MB], f32)
            nc.sync.dma_start(ln_sb[:], lnp[:])
            ones_sb = pt.tile([P, P], bf16)
            nc.vector.memset(ones_sb[:], 1.0)
            eps_sb = pt.tile([P, 1], f32)
            nc.vector.memset(eps_sb[:], LN_EPS)

            def layer_norm(src, lni, name):
                # stats summed over features AND broadcast to all partitions
                # via all-ones [128,128] matmuls (keeps gpsimd free for AG)
                ps_s = ps_st.tile([P, TT], f32, name=f'{name}_s1', tag='stat1')
                for k in range(MB):
                    nc.tensor.matmul(ps_s[:], ones_sb[:], src[k][:],
                                     start=(k == 0), stop=(k == MB - 1))
                ps_q = ps_st.tile([P, TT], f32, name=f'{name}_s2', tag='stat2')
                sqs = []
                for k in range(MB):
                    sq = ptw.tile([P, TT], bf16, name=f'{name}_sq{k}', tag=f'sq{k}')
                    nc.scalar.activation(sq[:], src[k][:], AFT.Square)
                    sqs.append(sq)
                for k in range(MB):
                    nc.tensor.matmul(ps_q[:], ones_sb[:], sqs[k][:],
                                     start=(k == 0), stop=(k == MB - 1))
                mu = ptw.tile([P, TT], f32, name=f'{name}_mu', tag='mu')
                nc.vector.tensor_scalar(mu[:], ps_s[:], 1.0 / D_MODEL, None,
                                        op0=Alu.mult)
                var = ptw.tile([P, TT], f32, name=f'{name}_var', tag='var')
                nc.vector.tensor_tensor(var[:], mu[:], mu[:], op=Alu.mult)
                nc.vector.scalar_tensor_tensor(
                    var[:], ps_q[:], 1.0 / D_MODEL, var[:],
                    op0=Alu.mult, op1=Alu.subtract)
                # rstd = exp(-0.5*ln(var+eps)) -- two ACT LUT ops instead of
                # the slow DVE iterative reciprocal
                lv = ptw.tile([P, TT], f32, name=f'{name}_lv', tag='lv')
                nc.scalar.activation(lv[:], var[:], AFT.Ln,
                                     bias=eps_sb[:], scale=1.0)
                rstd = ptw.tile([P, TT], bf16, name=f'{name}_rstd', tag='rstd')
                nc.scalar.activation(rstd[:], lv[:], AFT.Exp,
                                     bias=0.0, scale=-0.5)
                mu16 = ptw.tile([P, TT], bf16, name=f'{name}_mu16', tag='mu16')
                nc.vector.tensor_copy(mu16[:], mu[:])
                outs = []
                for k in range(MB):
                    o = ptw.tile([P, TT], bf16, name=f'{name}_o{k}',
                                 tag=f'{name}_o{k}')
                    nc.vector.tensor_tensor(o[:], src[k][:], mu16[:],
                                            op=Alu.subtract)
                    nc.vector.tensor_tensor(o[:], o[:], rstd[:], op=Alu.mult)
                    nc.vector.tensor_scalar(
                        o[:], o[:],
                        ln_sb[:, (2 * lni) * MB + k:(2 * lni) * MB + k + 1],
                        ln_sb[:, (2 * lni + 1) * MB + k:(2 * lni + 1) * MB + k + 1],
                        op0=Alu.mult, op1=Alu.add)
                    outs.append(o)
                return outs

            def ffn(src, w1l, b1t, w2l, b2t, name):
                f1 = []
                for fb in range(FB):
                    ps_f = pt_ps.tile([P, TT], f32, name=f'{name}_f{fb}', tag='ffp')
                    for kb in range(MB):
                        nc.tensor.matmul(ps_f[:], w1l[kb][:, ts(fb, P)], src[kb][:],
                                         start=(kb == 0), stop=(kb == MB - 1))
                    r = ptw.tile([P, TT], bf16, name=f'{name}_r{fb}', tag=f'ffr{fb}')
                    nc.scalar.activation(r[:], ps_f[:], AFT.Relu,
                                         bias=b1t[:, fb:fb + 1], scale=1.0)
                    f1.append(r)
                outs = []
                for mb in range(MB):
                    ps_g = pt_ps.tile([P, TT], f32, name=f'{name}_g{mb}', tag='ffq')
                    for kb in range(FB):
                        nc.tensor.matmul(ps_g[:], w2l[kb][:, ts(mb, P)], f1[kb][:],
                                         start=(kb == 0), stop=(kb == FB - 1))
                    o = ptw.tile([P, TT], f32, name=f'{name}_o{mb}', tag=f'ffo{mb}')
                    nc.vector.tensor_scalar(o[:], ps_g[:], 1.0, b2t[:, mb:mb + 1],
                                            op0=Alu.mult, op1=Alu.add)
                    outs.append(o)
                return outs

            def dump(nm, tiles, tqs):
                if not dbg:
                    return
                for k in range(MB):
                    tt_ = ptw.tile([P, TT], f32, name=f'dmp_{nm}_{k}', tag='dmp')
                    nc.vector.tensor_copy(tt_[:], tiles[k][:])
                    nc.sync.dma_start(dbg_tails[nm][ts(k, P), tqs], tt_[:])

            for tq in range(NTQ):
                tqs = ds(tq * TT, TT)
                # x_f branch reads own rs_out (valid on f-cores; b-cores'
                # tail result is discarded by host) -> overlaps the AllGather
                r1 = []
                for k in range(MB):
                    txf = ptw.tile([P, TT], f32, name=f'txf{k}', tag=f'txf{k}')
                    nc.sync.dma_start(txf[:], tail_x[ts(k, P), tqs])
                    xf = ptw.tile([P, TT], bf16, name=f'xf{k}', tag=f'xf{k}')
                    nc.sync.dma_start(xf[:], rs_out[ts(k, P), tqs])
                    a = ptw.tile([P, TT], bf16, name=f'r1_{k}', tag=f'r1_{k}')
                    nc.vector.tensor_tensor(a[:], xf[:], txf[:], op=Alu.add)
                    r1.append(a)
                dump('r1', r1, tqs)
                t1 = layer_norm(r1, 0, 'ln1')
                dump('t1', t1, tqs)
                ff1 = ffn(t1, w11, b11_sb, w12, b12_sb, 'ffn1')
                dump('ff1', ff1, tqs)
                s2 = []
                for k in range(MB):
                    s_ = ptw.tile([P, TT], bf16, name=f's2_{k}', tag=f's2_{k}')
                    nc.vector.tensor_tensor(s_[:], ff1[k][:], t1[k][:], op=Alu.add)
                    s2.append(s_)
                t2 = layer_norm(s2, 1, 'ln2')
                dump('t2', t2, tqs)
                ff2 = ffn(t2, w21, b21_sb, w22, b22_sb, 'ffn2')
                dump('ff2', ff2, tqs)
                # x_b branch: ag_out[1] holds partner's half in reversed token
                # order; un-flip on-chip with a reversed-AP DVE copy
                r3 = []
                for k in range(MB):
                    txf2 = ptw.tile([P, TT], f32, name=f'txg{k}', tag=f'txf{k}')
                    nc.sync.dma_start(txf2[:], tail_x[ts(k, P), tqs])
                    xbr = ptw.tile([P, TT], bf16, name=f'xbr{k}', tag=f'xbr{k}')
                    nc.sync.dma_start(
                        xbr[:], ag_out[1, ts(k, P), ds(TAIL - (tq + 1) * TT, TT)])
                    bt = ptw.tile([P, TT], bf16, name=f'r3_{k}', tag=f'r3_{k}')
                    nc.vector.tensor_tensor(bt[:], xbr[:, ::-1], txf2[:],
                                            op=Alu.add)
                    r3.append(bt)
                dump('r3', r3, tqs)
                t3 = layer_norm(r3, 2, 'ln3')
                dump('t3', t3, tqs)
                s4 = []
                for k in range(MB):
                    s_ = ptw.tile([P, TT], bf16, name=f's4_{k}', tag=f's4_{k}')
                    nc.vector.tensor_tensor(s_[:], ff2[k][:], t3[k][:], op=Alu.add)
                    s4.append(s_)
                t4 = layer_norm(s4, 3, 'ln4')
                dump('t4', t4, tqs)
                for k in range(MB):
                    o = ptw.tile([P, TT], f32, name=f'fin{k}', tag=f'fin{k}')
                    nc.vector.tensor_tensor(o[:], t2[k][:], t4[k][:], op=Alu.add)
                    nc.sync.dma_start(out_t[ts(k, P), tqs], o[:])

        dram_cm.__exit__(None, None, None)

    nc.compile()
    return nc


def _prep_inputs(inputs, L):
    """Build per-core in_maps from the full problem inputs."""
    TAIL = L // 2
    x = np.asarray(inputs['x'])
    in_maps = []
    for c in range(NCORES):
        b, rem = divmod(c, 4)
        dire, dh = divmod(rem, 2)
        p = 'f' if dire == 0 else 'b'
        dsl = slice(dh * DH, (dh + 1) * DH)
        xs = x[b] if dire == 0 else x[b][::-1]
        m = {}
        m['xT16'] = np.ascontiguousarray(xs.T).astype(ml_dtypes.bfloat16)
        in_w = np.asarray(inputs[p + '_in_w'])
        w_xz = np.concatenate([in_w[dsl], in_w[D_INNER:][dsl]], axis=0)  # [1024,512]
        m['in_wT'] = np.ascontiguousarray(w_xz.T).astype(ml_dtypes.bfloat16)
        cw = np.asarray(inputs[p + '_conv_w'])[dsl, 0, :]                # [512,4]
        m['conv_w'] = np.ascontiguousarray(
            cw.reshape(NB, P, D_CONV).transpose(1, 0, 2).reshape(P, NB * D_CONV)
        ).astype(np.float32)
        m['conv_b'] = np.ascontiguousarray(
            np.asarray(inputs[p + '_conv_b'])[dsl].reshape(NB, P).T
        ).astype(np.float32)
        xp = np.asarray(inputs[p + '_xproj_w'])[:, dsl]                  # [64,512]
        m['xproj_wT'] = np.ascontiguousarray(xp.T).astype(ml_dtypes.bfloat16)
        dtw = np.asarray(inputs[p + '_dt_w'])[dsl]                       # [512,32]
        m['dt_wT'] = np.ascontiguousarray(dtw.T).astype(ml_dtypes.bfloat16)
        m['dt_b'] = np.ascontiguousarray(
            np.asarray(inputs[p + '_dt_b'])[dsl].reshape(NB, P).T
        ).astype(np.float32)
        A = -np.exp(np.asarray(inputs[p + '_A_log'])[dsl])               # [512,16]
        m['A_sc'] = np.ascontiguousarray(
            A.reshape(NB, P, NSTATE).transpose(1, 0, 2).reshape(P, NB * NSTATE)
        ).astype(np.float32)
        m['D_in'] = np.ascontiguousarray(
            np.asarray(inputs[p + '_D'])[dsl].reshape(NB, P).T
        ).astype(np.float32)
        ow = np.asarray(inputs[p + '_out_w'])[:, dsl]                    # [512,512]
        m['out_wT'] = np.ascontiguousarray(ow.T).astype(ml_dtypes.bfloat16)
        for nm, key in (('f1w1', 'ffn1_w1'), ('f1w2', 'ffn1_w2'),
                        ('f2w1', 'ffn2_w1'), ('f2w2', 'ffn2_w2')):
            w = np.asarray(inputs[key])
            m[nm] = np.ascontiguousarray(w.T).astype(ml_dtypes.bfloat16)
        for nm, key, n_el in (('f1b1', 'ffn1_b1', D_FF), ('f1b2', 'ffn1_b2', D_MODEL),
                              ('f2b1', 'ffn2_b1', D_FF), ('f2b2', 'ffn2_b2', D_MODEL)):
            v = np.asarray(inputs[key]).reshape(n_el // P, P).T
            m[nm] = np.ascontiguousarray(v).astype(np.float32)
        ln = []
        for i in (1, 2, 3, 4):
            for sfx in ('w', 'b'):
                v = np.asarray(inputs[f'ln{i}_{sfx}']).reshape(D_MODEL // P, P).T
                ln.append(v)
        m['lnp'] = np.ascontiguousarray(np.concatenate(ln, axis=1)).astype(np.float32)
        th = dh ^ dire
        m['tail_x'] = np.ascontiguousarray(
            x[b, th * TAIL:(th + 1) * TAIL].T).astype(np.float32)
        in_maps.append(m)
    return in_maps


_PROGRAM_CACHE = {}


def kernel(**inputs):
    L = np.asarray(inputs['x']).shape[1]
    T = min(512, L // 2)
    key = (L, T)
    if key not in _PROGRAM_CACHE:
        _PROGRAM_CACHE[key] = build_program(L, T)
    nc = _PROGRAM_CACHE[key]
    in_maps = _prep_inputs(inputs, L)
    trace = os.environ.get('BIMAMBA_TRACE', '0') == '1'
    if trace:
        try:
            import ntff_shim
            ntff_shim.install()
        except Exception:
            trace = False
    res = run_bass_kernel_spmd(nc, in_maps, list(range(NCORES)), trace=trace)
    if trace and res.exec_time_ns is not None:
        kernel.last_exec_time_ns = res.exec_time_ns
    TAIL = L // 2
    x = np.asarray(inputs['x'])
    B = x.shape[0]
    out = np.empty((B, L, D_MODEL), np.float32)
    for b in range(B):
        out[b, 0:TAIL] = res.results[b * 4 + 0]['out'].T
        out[b, TAIL:L] = res.results[b * 4 + 1]['out'].T
    return out


kernel.last_exec_time_ns = None


# revision 34
# speedup vs baseline: 1.3145x; 1.0044x over previous
"""BiMamba encoder layer on 8 Trainium2 NeuronCores (Bass/Tile, SPMD).

Sharding: core c = (batch b, direction dir, d_inner-half dh), c = b*4 + dir*2 + dh.
All 8 cores run an IDENTICAL program; per-core behavior comes only from input
data (host pre-sliced/pre-flipped weights and activations) and collective
replica groups.

v2: all DRAM bounce buffers are FEATURE-MAJOR so every DMA moves contiguous
2KB rows (v1 used token-major transposing DMAs: 2.6M 4-byte packets, 96% DMA
time). The b-direction time-flip is done on-chip with reversed-AP DVE copies;
B/C state broadcasts use gpsimd partition_broadcast instead of DRAM broadcast
DMAs; the final [D_MODEL, TAIL] -> [TAIL, D_MODEL] transpose happens on host.

Pipeline per core (512 of 1024 d_inner channels, full L for one (b, dir)):
  A) in_proj (PE, bf16) -> causal conv (STT taps) -> silu; silu(z) saved;
     x_proj partial (PE) -> per-chunk pair AllReduce (overlapped)
  B) dt (PE) -> softplus (ACT) -> dA = exp(A*delta) (ACT) -> dBu (merged DVE
     mult w/ stride-0 broadcast) -> 16 hardware scans (DVE, in-place) ->
     y = C.h (DVE mul + tree) -> gate -> out_proj partial (PE) -> xm
     feature-major [2, 512, 1024] (l'-half major)
  C) pair ReduceScatter (l' halves) -> cross AllGather
  D) tail: x_f branch (r1/LN1/FFN1/LN2/FFN2) reads own rs_out (overlaps AG);
     x_b branch (LN3/LN4/sum) reads ag_out[1] reversed on-chip. LN stats are
     broadcast via ones-matmul on PE (no gpsimd -> no queue conflict with AG).

Host: out[b, th half] = f-direction cores' [D_MODEL, TAIL] output, transposed.
"""
import sys
import os

sys.path.insert(0, '/opt/trn_rl_repo')

import numpy as np
import ml_dtypes

import concourse.bass as bass
import concourse.mybir as mybir
import concourse.tile as tile
from concourse import bacc
from concourse.bass_utils import run_bass_kernel_spmd
from concourse.bass import ds, ts

f32 = mybir.dt.float32
bf16 = mybir.dt.bfloat16
Alu = mybir.AluOpType
AFT = mybir.ActivationFunctionType

P = 128
D_MODEL = 512
D_INNER = 1024
DH = 512            # d_inner channels per core
NB = DH // P        # 4 channel blocks per core
NSTATE = 16
DT_RANK = 32
DBC = DT_RANK + 2 * NSTATE   # 64
D_CONV = 4
D_FF = 1024
NCORES = 8
LN_EPS = 1e-5

PAIRS = [[0, 1], [2, 3], [4, 5], [6, 7]]
CROSS = [[0, 3], [1, 2], [4, 7], [5, 6]]


def build_program(L, T, dbg=False):
    """Emit the SPMD program for sequence length L, phase chunk T."""
    NCH = L // T
    TAIL = L // 2
    MB = D_MODEL // P   # 4 blocks of d_model
    FB = D_FF // P      # 8 blocks of d_ff
    NT = NSTATE * T

    nc = bacc.Bacc('TRN2', target_bir_lowering=False, debug=False,
                   num_devices=NCORES)

    def din(name, shape, dt=f32):
        return nc.dram_tensor(name, shape, dt, kind='ExternalInput')

    xT16 = din('xT16', [D_MODEL, L], bf16)
    in_wT = din('in_wT', [D_MODEL, 2 * DH], bf16)       # K x (xi|z)
    conv_w = din('conv_w', [P, NB * D_CONV])
    conv_b = din('conv_b', [P, NB])
    xproj_wT = din('xproj_wT', [DH, DBC], bf16)
    dt_wT = din('dt_wT', [DT_RANK, DH], bf16)
    dt_b = din('dt_b', [P, NB])
    A_sc = din('A_sc', [P, NB * NSTATE])
    D_in = din('D_in', [P, NB])
    out_wT = din('out_wT', [DH, D_MODEL], bf16)
    f1w1 = din('f1w1', [D_MODEL, D_FF], bf16)
    f1b1 = din('f1b1', [P, D_FF // P])
    f1w2 = din('f1w2', [D_FF, D_MODEL], bf16)
    f1b2 = din('f1b2', [P, D_MODEL // P])
    f2w1 = din('f2w1', [D_MODEL, D_FF], bf16)
    f2b1 = din('f2b1', [P, D_FF // P])
    f2w2 = din('f2w2', [D_FF, D_MODEL], bf16)
    f2b2 = din('f2b2', [P, D_MODEL // P])
    lnp = din('lnp', [P, 8 * (D_MODEL // P)])           # ln1..4 w,b
    tail_x = din('tail_x', [D_MODEL, TAIL])
    out_t = nc.dram_tensor('out', [D_MODEL, TAIL], f32, kind='ExternalOutput')
    if dbg:
        dbg_xic = nc.dram_tensor('dbg_xic', [DH, L], f32, kind='ExternalOutput')
        dbg_sz = nc.dram_tensor('dbg_sz', [DH, L], f32, kind='ExternalOutput')
        dbg_dbc = nc.dram_tensor('dbg_dbc', [DBC, L], f32, kind='ExternalOutput')
        dbg_delta = nc.dram_tensor('dbg_delta', [P, T], f32, kind='ExternalOutput')
        dbg_h = nc.dram_tensor('dbg_h', [P, NSTATE * T], f32, kind='ExternalOutput')
        dbg_yg = nc.dram_tensor('dbg_yg', [P, T], f32, kind='ExternalOutput')
        dbg_xm = nc.dram_tensor('dbg_xm', [D_MODEL, L], bf16, kind='ExternalOutput')
        dbg_rs = nc.dram_tensor('dbg_rs', [D_MODEL, TAIL], bf16, kind='ExternalOutput')
        dbg_ag = nc.dram_tensor('dbg_ag', [2 * D_MODEL, TAIL], bf16,
                                kind='ExternalOutput')
        dbg_tails = {_n: nc.dram_tensor('dbg_' + _n, [D_MODEL, TAIL], f32,
                                        kind='ExternalOutput')
                     for _n in ('r1', 't1', 'ff1', 't2', 'r3', 't3', 'ff2', 't4')}

    with tile.TileContext(nc) as tc:
        dram_cm = tc.tile_pool(name='dram', bufs=1, space='DRAM')
        dram = dram_cm.__enter__()
        dbc_bo = dram.tile([NCH, DBC, T], f32)
        dbc_ar = dram.tile([NCH, DBC, T], f32)
        dbcBC = dram.tile([NCH, 2 * NSTATE, T], bf16)
        xm_bo = dram.tile([2, D_MODEL, TAIL], bf16)
        rs_out = dram.tile([D_MODEL, TAIL], bf16)
        ag_out = dram.tile([2, D_MODEL, TAIL], bf16)

        with tc.tile_pool(name='pers', bufs=1) as pers:
            # persistent SBUF (live through phases A-B; small ones to D)
            xpw_sb = [pers.tile([P, DBC], bf16, name=f'xpw{k}')
                      for k in range(NB)]
            for k in range(NB):
                nc.sync.dma_start(xpw_sb[k][:], xproj_wT[ts(k, P), :])
            dtw_sb = pers.tile([DT_RANK, DH], bf16)
            nc.sync.dma_start(dtw_sb[:], dt_wT[:])
            outw_sb = [pers.tile([P, D_MODEL], bf16, name=f'outw{k}') for k in range(NB)]
            for k in range(NB):
                nc.sync.dma_start(outw_sb[k][:], out_wT[ts(k, P), :])
            cw_sb = pers.tile([P, NB * D_CONV], f32)
            nc.sync.dma_start(cw_sb[:], conv_w[:])
            cb_sb = pers.tile([P, NB], f32)
            nc.sync.dma_start(cb_sb[:], conv_b[:])
            dtb_sb = pers.tile([P, NB], f32)
            nc.sync.dma_start(dtb_sb[:], dt_b[:])
            asc_sb = pers.tile([P, NB * NSTATE], f32)
            nc.sync.dma_start(asc_sb[:], A_sc[:])
            d_sb = pers.tile([P, NB], f32)
            nc.sync.dma_start(d_sb[:], D_in[:])

            xi_c = [pers.tile([P, L], bf16, name=f'xic{k}') for k in range(NB)]
            sz = [pers.tile([P, L], bf16, name=f'sz{k}') for k in range(NB)]
            halo = [pers.tile([P, D_CONV - 1], bf16, name=f'halo{k}') for k in range(NB)]
            for k in range(NB):
                nc.vector.memset(halo[k][:], 0.0)
            dbc16 = pers.tile([DBC, L], bf16)

            # ---------------- Phase A ----------------
            with tc.tile_pool(name='pax', bufs=1) as pax, \
                 tc.tile_pool(name='pa', bufs=3) as pa, \
                 tc.tile_pool(name='pa_ps', bufs=4, space='PSUM') as pa_ps:
                inw_sb = [pax.tile([P, 2 * DH], bf16, name=f'inw{k}')
                          for k in range(MB)]
                for k in range(MB):
                    nc.sync.dma_start(inw_sb[k][:], in_wT[ts(k, P), :])
                # x loaded per chunk so chunk 0's in_proj starts ASAP
                x_sb = [pax.tile([P, L], bf16, name=f'x_sb{k}') for k in range(MB)]
                for ci in range(NCH):
                    for k in range(MB):
                        nc.sync.dma_start(x_sb[k][:, ds(ci * T, T)],
                                          xT16[ts(k, P), ds(ci * T, T)])
                for ci in range(NCH):
                    tsl = ds(ci * T, T)
                    for mb in range(2 * NB):   # 4 xi blocks then 4 z blocks
                        ps_t = pa_ps.tile([P, T], f32, name='inproj')
                        for kb in range(MB):
                            nc.tensor.matmul(
                                ps_t[:], inw_sb[kb][:, ts(mb, P)],
                                x_sb[kb][:, tsl],
                                start=(kb == 0), stop=(kb == MB - 1))
                        if mb < NB:
                            db = mb
                            xr = pa.tile([P, D_CONV - 1 + T], bf16, name='xr',
                                         tag='xr')
                            nc.vector.tensor_copy(xr[:, 0:D_CONV - 1], halo[db][:])
                            nc.scalar.activation(xr[:, D_CONV - 1:], ps_t[:],
                                                 AFT.Copy)
                            nc.vector.tensor_copy(halo[db][:], xr[:, T:])
                            # causal conv: tap 0 with bias, then taps 1..3
                            xc = pa.tile([P, T], bf16, name='xc', tag='xc')
                            nc.vector.tensor_scalar(
                                xc[:], xr[:, 0:T],
                                cw_sb[:, db * D_CONV:db * D_CONV + 1],
                                cb_sb[:, db:db + 1],
                                op0=Alu.mult, op1=Alu.add)
                            for k in range(1, D_CONV):
                                nc.vector.scalar_tensor_tensor(
                                    xc[:], xr[:, k:k + T],
                                    cw_sb[:, db * D_CONV + k:db * D_CONV + k + 1],
                                    xc[:], op0=Alu.mult, op1=Alu.add)
                            nc.scalar.activation(xi_c[db][:, tsl], xc[:], AFT.Silu)
                        else:
                            # raw z copied on DVE (phase A ACT is the
                            # head-of-line for phase B's delta); silu applied
                            # in the consume stage on ACT
                            db = mb - NB
                            nc.vector.tensor_copy(sz[db][:, tsl], ps_t[:])
                    # x_proj partial for this chunk -> per-chunk AllReduce
                    ps_x = pa_ps.tile([P, T], f32, name='xproj')
                    for kb in range(NB):
                        nc.tensor.matmul(
                            ps_x[0:DBC, :], xpw_sb[kb][:],
                            xi_c[kb][:, tsl],
                            start=(kb == 0), stop=(kb == NB - 1))
                    dbc_p = pa.tile([DBC, T], f32, name='dbcp', tag='dbcp')
                    nc.scalar.activation(dbc_p[:], ps_x[0:DBC, :], AFT.Copy)
                    nc.sync.dma_start(dbc_bo[ci], dbc_p[:])
                    nc.gpsimd.collective_compute(
                        'AllReduce', Alu.add, replica_groups=PAIRS,
                        ins=[dbc_bo[ci].opt()], outs=[dbc_ar[ci].opt()])



            if dbg:
                with tc.tile_pool(name='dbgp', bufs=1) as dbgp:
                    for k in range(NB):
                        t_ = dbgp.tile([P, L], f32, name='dbgt', tag='dbgt')
                        nc.vector.tensor_copy(t_[:], xi_c[k][:])
                        nc.sync.dma_start(dbg_xic[ts(k, P), :], t_[:])
                        t2_ = dbgp.tile([P, L], f32, name='dbgt2', tag='dbgt')
                        nc.vector.tensor_copy(t2_[:], sz[k][:])
                        nc.sync.dma_start(dbg_sz[ts(k, P), :], t2_[:])
                    t3_ = dbgp.tile([DBC, L], f32, name='dbgt3', tag='dbgt3')
                    nc.vector.tensor_copy(t3_[:], dbc16[:])
                    nc.sync.dma_start(dbg_dbc[:], t3_[:])

            # ---------------- Phase B ----------------
            state = [pers.tile([P, NSTATE], f32, name=f'st{k}') for k in range(NB)]
            with tc.tile_pool(name='pb', bufs=2) as pb, \
                 tc.tile_pool(name='pb0', bufs=2) as pb0, \
                 tc.tile_pool(name='pbh', bufs=2) as pbh, \
                 tc.tile_pool(name='pbr', bufs=2) as pbr, \
                 tc.tile_pool(name='pb_ps', bufs=2, space='PSUM') as pb_ps, \
                 tc.tile_pool(name='po_ps', bufs=1, space='PSUM') as po_ps:
                def consume(h, ci, db, tsl, ps_o):
                    half = NSTATE
                    while half > 1:
                        half //= 2
                        nc.vector.tensor_tensor(
                            h[:, 0:half * T], h[:, 0:half * T],
                            h[:, half * T:2 * half * T], op=Alu.add)
                    yg = pb.tile([P, T], bf16, name='yg', tag='yg')
                    nc.vector.scalar_tensor_tensor(
                        yg[:], xi_c[db][:, tsl], d_sb[:, db:db + 1],
                        h[:, 0:T], op0=Alu.mult, op1=Alu.add)
                    nc.vector.tensor_tensor(yg[:], yg[:], sgates[db][:],
                                            op=Alu.mult)
                    if dbg and ci == 0 and db == 0:
                        ygf = pb.tile([P, T], f32, name='ygf', tag='ygf')
                        nc.vector.tensor_copy(ygf[:], yg[:])
                        nc.sync.dma_start(dbg_yg[:], ygf[:])
                    for mb in range(MB):
                        nc.tensor.matmul(
                            ps_o[mb][:], outw_sb[db][:, ts(mb, P)], yg[:],
                            start=(db == 0), stop=(db == NB - 1))
                    if db == NB - 1:
                        hf_, off = divmod(ci * T, TAIL)
                        for mb in range(MB):
                            xm_sb = pb.tile([P, T], bf16, name='xm', tag='xm')
                            nc.scalar.activation(xm_sb[:], ps_o[mb][:], AFT.Copy)
                            nc.sync.dma_start(
                                xm_bo[hf_, ts(mb, P), ds(off, T)], xm_sb[:])

                for ci in range(NCH):
                    tsl = ds(ci * T, T)
                    # dbc AllReduce landing: load + bf16 convert + broadcast
                    # staging, per chunk (keeps DVE queue free of later-chunk
                    # AR waits -- head-of-line fix)
                    dbc_f = pb0.tile([DBC, T], f32, name='dbcf', tag='dbcf')
                    nc.sync.dma_start(dbc_f[:], dbc_ar[ci])
                    nc.vector.tensor_copy(dbc16[:, tsl], dbc_f[:])
                    nc.sync.dma_start(dbcBC[ci], dbc16[DT_RANK:DBC, tsl])
                    b_rep = pbr.tile([P, NT], bf16, name='b_rep', tag='b_rep')
                    c_rep = pbr.tile([P, NT], bf16, name='c_rep', tag='c_rep')
                    bc_flat = dbcBC[ci].rearrange('n t -> (n t)').unsqueeze(0)
                    nc.sync.dma_start(
                        b_rep[:], bc_flat[:, 0:NT].broadcast_to([P, NT]))
                    nc.sync.dma_start(
                        c_rep[:], bc_flat[:, NT:2 * NT].broadcast_to([P, NT]))
                    # all 4 blocks' silu(z) grouped -> one ACT table switch
                    # per chunk instead of two per (ci, db) iteration
                    sgates = []
                    for db in range(NB):
                        sg = pb.tile([P, T], bf16, name=f'sg{db}', tag=f'sg{db}')
                        nc.scalar.activation(sg[:], sz[db][:, tsl], AFT.Silu)
                        sgates.append(sg)
                    ps_o = [po_ps.tile([P, T], f32, name=f'pso{mb}',
                                       tag=f'pso{mb}') for mb in range(MB)]
                    for db in range(NB):
                        ps_d = pb_ps.tile([P, T], f32, name='dt', tag='dt')
                        nc.tensor.matmul(ps_d[:], dtw_sb[:, ts(db, P)],
                                         dbc16[0:DT_RANK, tsl],
                                         start=True, stop=True)
                        delta = pb.tile([P, T], f32, name='delta', tag='delta')
                        nc.scalar.activation(delta[:], ps_d[:], AFT.Exp,
                                             bias=dtb_sb[:, db:db + 1], scale=1.0)
                        nc.scalar.activation(delta[:], delta[:], AFT.Ln,
                                             bias=1.0, scale=1.0)
                        d16 = pb.tile([P, T], bf16, name='d16', tag='d16')
                        nc.vector.tensor_copy(d16[:], delta[:])
                        dA = pb.tile([P, NT], bf16, name='dA', tag='dA')
                        for n in range(NSTATE):
                            nc.scalar.activation(
                                dA[:, ts(n, T)], d16[:], AFT.Exp,
                                bias=0.0,
                                scale=asc_sb[:, db * NSTATE + n:db * NSTATE + n + 1])
                        if dbg and ci == 0 and db == 0:
                            nc.sync.dma_start(dbg_delta[:], delta[:])
                        du = pb.tile([P, T], bf16, name='du', tag='du')
                        nc.vector.tensor_tensor(du[:], d16[:], xi_c[db][:, tsl],
                                                op=Alu.mult)
                        # merged dBu: du broadcast over states via stride-0 dim
                        h = pbh.tile([P, NT], bf16, name='h', tag='h')
                        nc.vector.tensor_tensor(
                            h[:].rearrange('p (n t) -> p n t', n=NSTATE),
                            du[:].unsqueeze(1).broadcast_to([P, NSTATE, T]),
                            b_rep[:].rearrange('p (n t) -> p n t', n=NSTATE),
                            op=Alu.mult)
                        for n in range(NSTATE):
                            nc.vector.tensor_tensor_scan(
                                h[:, ts(n, T)], dA[:, ts(n, T)], h[:, ts(n, T)],
                                0.0 if ci == 0 else state[db][:, n:n + 1],
                                op0=Alu.mult, op1=Alu.add)
                        nc.vector.tensor_copy(
                            state[db][:],
                            h.rearrange('p (n t) -> p n t', n=NSTATE)[:, :, T - 1])
                        if dbg and ci == 0 and db == 0:
                            hf = pb.tile([P, NSTATE * T], f32, name='hf', tag='hf')
                            nc.vector.tensor_copy(hf[:], h[:])
                            nc.sync.dma_start(dbg_h[:], hf[:])
                        # y = sum_n C_n * h_n (gpsimd shares the DVE SBUF port,
                        # so offloading this mul to gpsimd slows DVE more than
                        # it saves -- measured; keep everything on DVE)
                        nc.vector.tensor_tensor(h[:], h[:], c_rep[:], op=Alu.mult)
                        consume(h, ci, db, tsl, ps_o)

            # ---------------- Phase C: collectives ----------------
            if dbg:
                nc.sync.dma_start(dbg_xm[:, 0:TAIL], xm_bo[0])
                nc.sync.dma_start(dbg_xm[:, TAIL:], xm_bo[1])
            nc.gpsimd.collective_compute(
                'ReduceScatter', Alu.add, replica_groups=PAIRS,
                ins=[xm_bo.opt()], outs=[rs_out.opt()])
            nc.gpsimd.collective_compute(
                'AllGather', Alu.bypass, replica_groups=CROSS,
                ins=[rs_out.opt()], outs=[ag_out.opt()])
            if dbg:
                nc.sync.dma_start(dbg_rs[:], rs_out[:])
                nc.sync.dma_start(dbg_ag[0:D_MODEL, :], ag_out[0])
                nc.sync.dma_start(dbg_ag[D_MODEL:, :], ag_out[1])

        # ---------------- Phase D: tail (slabs of <=512 tokens) ----------------
        TT = min(512, TAIL)
        NTQ = TAIL // TT
        with tc.tile_pool(name='pt', bufs=1) as pt, \
             tc.tile_pool(name='ptw', bufs=2) as ptw, \
             tc.tile_pool(name='pt_ps', bufs=2, space='PSUM') as pt_ps, \
             tc.tile_pool(name='ps_st', bufs=2, space='PSUM') as ps_st:
            w11 = [pt.tile([P, D_FF], bf16, name=f'w11_{k}') for k in range(MB)]
            for k in range(MB):
                nc.sync.dma_start(w11[k][:], f1w1[ts(k, P), :])
            w12 = [pt.tile([P, D_MODEL], bf16, name=f'w12_{k}') for k in range(FB)]
            for k in range(FB):
                nc.sync.dma_start(w12[k][:], f1w2[ts(k, P), :])
            w21 = [pt.tile([P, D_FF], bf16, name=f'w21_{k}') for k in range(MB)]
            for k in range(MB):
                nc.sync.dma_start(w21[k][:], f2w1[ts(k, P), :])
            w22 = [pt.tile([P, D_MODEL], bf16, name=f'w22_{k}') for k in range(FB)]
            for k in range(FB):
                nc.sync.dma_start(w22[k][:], f2w2[ts(k, P), :])
            b11_sb = pt.tile([P, FB], f32)
            nc.sync.dma_start(b11_sb[:], f1b1[:])
            b12_sb = pt.tile([P, MB], f32)
            nc.sync.dma_start(b12_sb[:], f1b2[:])
            b21_sb = pt.tile([P, FB], f32)
            nc.sync.dma_start(b21_sb[:], f2b1[:])
            b22_sb = pt.tile([P, MB], f32)
            nc.sync.dma_start(b22_sb[:], f2b2[:])
            ln_sb = pt.tile([P, 8 * MB], f32)
            nc.sync.dma_start(ln_sb[:], lnp[:])
            ones_sb = pt.tile([P, P], bf16)
            nc.vector.memset(ones_sb[:], 1.0)
            eps_sb = pt.tile([P, 1], f32)
            nc.vector.memset(eps_sb[:], LN_EPS)

            def layer_norm(src, lni, name):
                # stats summed over features AND broadcast to all partitions
                # via all-ones [128,128] matmuls (keeps gpsimd free for AG)
                ps_s = ps_st.tile([P, TT], f32, name=f'{name}_s1', tag='stat1')
                for k in range(MB):
                    nc.tensor.matmul(ps_s[:], ones_sb[:], src[k][:],
                                     start=(k == 0), stop=(k == MB - 1))
                ps_q = ps_st.tile([P, TT], f32, name=f'{name}_s2', tag='stat2')
                sqs = []
                for k in range(MB):
                    sq = ptw.tile([P, TT], bf16, name=f'{name}_sq{k}', tag=f'sq{k}')
                    nc.scalar.activation(sq[:], src[k][:], AFT.Square)
                    sqs.append(sq)
                for k in range(MB):
                    nc.tensor.matmul(ps_q[:], ones_sb[:], sqs[k][:],
                                     start=(k == 0), stop=(k == MB - 1))
                mu = ptw.tile([P, TT], f32, name=f'{name}_mu', tag='mu')
                nc.vector.tensor_scalar(mu[:], ps_s[:], 1.0 / D_MODEL, None,
                                        op0=Alu.mult)
                var = ptw.tile([P, TT], f32, name=f'{name}_var', tag='var')
                nc.vector.tensor_tensor(var[:], mu[:], mu[:], op=Alu.mult)
                nc.vector.scalar_tensor_tensor(
                    var[:], ps_q[:], 1.0 / D_MODEL, var[:],
                    op0=Alu.mult, op1=Alu.subtract)
                # rstd = exp(-0.5*ln(var+eps)) -- two ACT LUT ops instead of
                # the slow DVE iterative reciprocal
                lv = ptw.tile([P, TT], f32, name=f'{name}_lv', tag='lv')
                nc.scalar.activation(lv[:], var[:], AFT.Ln,
                                     bias=eps_sb[:], scale=1.0)
                rstd = ptw.tile([P, TT], bf16, name=f'{name}_rstd', tag='rstd')
                nc.scalar.activation(rstd[:], lv[:], AFT.Exp,
                                     bias=0.0, scale=-0.5)
                mu16 = ptw.tile([P, TT], bf16, name=f'{name}_mu16', tag='mu16')
                nc.vector.tensor_copy(mu16[:], mu[:])
                outs = []
                for k in range(MB):
                    o = ptw.tile([P, TT], bf16, name=f'{name}_o{k}',
                                 tag=f'{name}_o{k}')
                    nc.vector.tensor_tensor(o[:], src[k][:], mu16[:],
                                            op=Alu.subtract)
                    nc.vector.tensor_tensor(o[:], o[:], rstd[:], op=Alu.mult)
                    nc.vector.tensor_scalar(
                        o[:], o[:],
                        ln_sb[:, (2 * lni) * MB + k:(2 * lni) * MB + k + 1],
                        ln_sb[:, (2 * lni + 1) * MB + k:(2 * lni + 1) * MB + k + 1],
                        op0=Alu.mult, op1=Alu.add)
                    outs.append(o)
                return outs

            def ffn(src, w1l, b1t, w2l, b2t, name):
                f1 = []
                for fb in range(FB):
                    ps_f = pt_ps.tile([P, TT], f32, name=f'{name}_f{fb}', tag='ffp')
                    for kb in range(MB):
                        nc.tensor.matmul(ps_f[:], w1l[kb][:, ts(fb, P)], src[kb][:],
                                         start=(kb == 0), stop=(kb == MB - 1))
                    r = ptw.tile([P, TT], bf16, name=f'{name}_r{fb}', tag=f'ffr{fb}')
                    nc.scalar.activation(r[:], ps_f[:], AFT.Relu,
                                         bias=b1t[:, fb:fb + 1], scale=1.0)
                    f1.append(r)
                outs = []
                for mb in range(MB):
                    ps_g = pt_ps.tile([P, TT], f32, name=f'{name}_g{mb}', tag='ffq')
                    for kb in range(FB):
                        nc.tensor.matmul(ps_g[:], w2l[kb][:, ts(mb, P)], f1[kb][:],
                                         start=(kb == 0), stop=(kb == FB - 1))
                    o = ptw.tile([P, TT], f32, name=f'{name}_o{mb}', tag=f'ffo{mb}')
                    nc.vector.tensor_scalar(o[:], ps_g[:], 1.0, b2t[:, mb:mb + 1],
                                            op0=Alu.mult, op1=Alu.add)
                    outs.append(o)
                return outs

            def dump(nm, tiles, tqs):
                if not dbg:
                    return
                for k in range(MB):
                    tt_ = ptw.tile([P, TT], f32, name=f'dmp_{nm}_{k}', tag='dmp')
                    nc.vector.tensor_copy(tt_[:], tiles[k][:])
                    nc.sync.dma_start(dbg_tails[nm][ts(k, P), tqs], tt_[:])

            for tq in range(NTQ):
                tqs = ds(tq * TT, TT)
                # x_f branch reads own rs_out (valid on f-cores; b-cores'
                # tail result is discarded by host) -> overlaps the AllGather
                r1 = []
                for k in range(MB):
                    txf = ptw.tile([P, TT], f32, name=f'txf{k}', tag=f'txf{k}')
                    nc.sync.dma_start(txf[:], tail_x[ts(k, P), tqs])
                    xf = ptw.tile([P, TT], bf16, name=f'xf{k}', tag=f'xf{k}')
                    nc.sync.dma_start(xf[:], rs_out[ts(k, P), tqs])
                    a = ptw.tile([P, TT], bf16, name=f'r1_{k}', tag=f'r1_{k}')
                    nc.vector.tensor_tensor(a[:], xf[:], txf[:], op=Alu.add)
                    r1.append(a)
                dump('r1', r1, tqs)
                t1 = layer_norm(r1, 0, 'ln1')
                dump('t1', t1, tqs)
                ff1 = ffn(t1, w11, b11_sb, w12, b12_sb, 'ffn1')
                dump('ff1', ff1, tqs)
                s2 = []
                for k in range(MB):
                    s_ = ptw.tile([P, TT], bf16, name=f's2_{k}', tag=f's2_{k}')
                    nc.vector.tensor_tensor(s_[:], ff1[k][:], t1[k][:], op=Alu.add)
                    s2.append(s_)
                t2 = layer_norm(s2, 1, 'ln2')
                dump('t2', t2, tqs)
                ff2 = ffn(t2, w21, b21_sb, w22, b22_sb, 'ffn2')
                dump('ff2', ff2, tqs)
                # x_b branch: ag_out[1] holds partner's half in reversed token
                # order; un-flip on-chip with a reversed-AP DVE copy
                r3 = []
                for k in range(MB):
                    txf2 = ptw.tile([P, TT], f32, name=f'txg{k}', tag=f'txf{k}')
                    nc.sync.dma_start(txf2[:], tail_x[ts(k, P), tqs])
                    xbr = ptw.tile([P, TT], bf16, name=f'xbr{k}', tag=f'xbr{k}')
                    nc.sync.dma_start(
                        xbr[:], ag_out[1, ts(k, P), ds(TAIL - (tq + 1) * TT, TT)])
                    bt = ptw.tile([P, TT], bf16, name=f'r3_{k}', tag=f'r3_{k}')
                    nc.vector.tensor_tensor(bt[:], xbr[:, ::-1], txf2[:],
                                            op=Alu.add)
                    r3.append(bt)
                dump('r3', r3, tqs)
                t3 = layer_norm(r3, 2, 'ln3')
                dump('t3', t3, tqs)
                s4 = []
                for k in range(MB):
                    s_ = ptw.tile([P, TT], bf16, name=f's4_{k}', tag=f's4_{k}')
                    nc.vector.tensor_tensor(s_[:], ff2[k][:], t3[k][:], op=Alu.add)
                    s4.append(s_)
                t4 = layer_norm(s4, 3, 'ln4')
                dump('t4', t4, tqs)
                for k in range(MB):
                    o = ptw.tile([P, TT], f32, name=f'fin{k}', tag=f'fin{k}')
                    nc.vector.tensor_tensor(o[:], t2[k][:], t4[k][:], op=Alu.add)
                    nc.sync.dma_start(out_t[ts(k, P), tqs], o[:])

        dram_cm.__exit__(None, None, None)

    nc.compile()
    return nc


def _prep_inputs(inputs, L):
    """Build per-core in_maps from the full problem inputs."""
    TAIL = L // 2
    x = np.asarray(inputs['x'])
    in_maps = []
    for c in range(NCORES):
        b, rem = divmod(c, 4)
        dire, dh = divmod(rem, 2)
        p = 'f' if dire == 0 else 'b'
        dsl = slice(dh * DH, (dh + 1) * DH)
        xs = x[b] if dire == 0 else x[b][::-1]
        m = {}
        m['xT16'] = np.ascontiguousarray(xs.T).astype(ml_dtypes.bfloat16)
        in_w = np.asarray(inputs[p + '_in_w'])
        w_xz = np.concatenate([in_w[dsl], in_w[D_INNER:][dsl]], axis=0)  # [1024,512]
        m['in_wT'] = np.ascontiguousarray(w_xz.T).astype(ml_dtypes.bfloat16)
        cw = np.asarray(inputs[p + '_conv_w'])[dsl, 0, :]                # [512,4]
        m['conv_w'] = np.ascontiguousarray(
            cw.reshape(NB, P, D_CONV).transpose(1, 0, 2).reshape(P, NB * D_CONV)
        ).astype(np.float32)
        m['conv_b'] = np.ascontiguousarray(
            np.asarray(inputs[p + '_conv_b'])[dsl].reshape(NB, P).T
        ).astype(np.float32)
        xp = np.asarray(inputs[p + '_xproj_w'])[:, dsl]                  # [64,512]
        m['xproj_wT'] = np.ascontiguousarray(xp.T).astype(ml_dtypes.bfloat16)
        dtw = np.asarray(inputs[p + '_dt_w'])[dsl]                       # [512,32]
        m['dt_wT'] = np.ascontiguousarray(dtw.T).astype(ml_dtypes.bfloat16)
        m['dt_b'] = np.ascontiguousarray(
            np.asarray(inputs[p + '_dt_b'])[dsl].reshape(NB, P).T
        ).astype(np.float32)
        A = -np.exp(np.asarray(inputs[p + '_A_log'])[dsl])               # [512,16]
        m['A_sc'] = np.ascontiguousarray(
            A.reshape(NB, P, NSTATE).transpose(1, 0, 2).reshape(P, NB * NSTATE)
        ).astype(np.float32)
        m['D_in'] = np.ascontiguousarray(
            np.asarray(inputs[p + '_D'])[dsl].reshape(NB, P).T
        ).astype(np.float32)
        ow = np.asarray(inputs[p + '_out_w'])[:, dsl]                    # [512,512]
        m['out_wT'] = np.ascontiguousarray(ow.T).astype(ml_dtypes.bfloat16)
        for nm, key in (('f1w1', 'ffn1_w1'), ('f1w2', 'ffn1_w2'),
                        ('f2w1', 'ffn2_w1'), ('f2w2', 'ffn2_w2')):
            w = np.asarray(inputs[key])
            m[nm] = np.ascontiguousarray(w.T).astype(ml_dtypes.bfloat16)
        for nm, key, n_el in (('f1b1', 'ffn1_b1', D_FF), ('f1b2', 'ffn1_b2', D_MODEL),
                              ('f2b1', 'ffn2_b1', D_FF), ('f2b2', 'ffn2_b2', D_MODEL)):
            v = np.asarray(inputs[key]).reshape(n_el // P, P).T
            m[nm] = np.ascontiguousarray(v).astype(np.float32)
        ln = []
        for i in (1, 2, 3, 4):
            for sfx in ('w', 'b'):
                v = np.asarray(inputs[f'ln{i}_{sfx}']).reshape(D_MODEL // P, P).T
                ln.append(v)
        m['lnp'] = np.ascontiguousarray(np.concatenate(ln, axis=1)).astype(np.float32)
        th = dh ^ dire
        m['tail_x'] = np.ascontiguousarray(
            x[b, th * TAIL:(th + 1) * TAIL].T).astype(np.float32)
        in_maps.append(m)
    return in_maps


_PROGRAM_CACHE = {}


def kernel(**inputs):
    L = np.asarray(inputs['x']).shape[1]
    T = min(512, L // 2)
    key = (L, T)
    if key not in _PROGRAM_CACHE:
        _PROGRAM_CACHE[key] = build_program(L, T)
    nc = _PROGRAM_CACHE[key]
    in_maps = _prep_inputs(inputs, L)
    trace = os.environ.get('BIMAMBA_TRACE', '0') == '1'
    if trace:
        try:
            import ntff_shim
            ntff_shim.install()
        except Exception:
            trace = False
    res = run_bass_kernel_spmd(nc, in_maps, list(range(NCORES)), trace=trace)
    if trace and res.exec_time_ns is not None:
        kernel.last_exec_time_ns = res.exec_time_ns
    TAIL = L // 2
    x = np.asarray(inputs['x'])
    B = x.shape[0]
    out = np.empty((B, L, D_MODEL), np.float32)
    for b in range(B):
        out[b, 0:TAIL] = res.results[b * 4 + 0]['out'].T
        out[b, TAIL:L] = res.results[b * 4 + 1]['out'].T
    return out


kernel.last_exec_time_ns = None
